# revision 32
# baseline (speedup 1.0000x reference)
"""Trainium2 Bass kernel for nn_MoEAdaptorLayer (whiten -> causal MHA -> proj
-> noisy-top-k gating (eval) -> 8 dense experts -> gated mixture * expert sum).

Sharding: data-parallel over batch. 64 batches -> 8 per core, params replicated.

v2 redesign vs baseline:
- fp16 everywhere on the matmul path (fp32 PSUM accumulation); halves DMA and
  removes the fp32r small-free-dim PE penalties.
- qkv: x shipped as one [768, 2400] fp16 tensor per core (6 big DMAs), matmuls
  in [128, 480] chunks.
- attention: single-orientation flash-style pass. Scores computed [s, t] only;
  exp on scalar; causal mask by multiplying the diagonal blocks with a
  triangular constant (split vector/gpsimd). AV and the softmax denominator
  come from ONE matmul per (head, s-chunk): stationary [v_h | ones] (32 cols)
  so the psum holds [av_h (16 rows) | Z_h replicated (16 rows)] per 32-stripe.
  One reciprocal + one fused min-mult STT per batch produce attnT directly in
  head-striped [128, tok] layout (no Z-pass, no output transposes).
- proj: stationary is zero-padded to the 32-stripe layout; 15 fp16 matmuls.
- gating: logits computed transposed ([8, tok], weight-stationary, 15 matmuls
  instead of 57 LDW-bound ones), PE-transposed back in 19 tiny transposes.
- experts: h chunked {128,128,128,128,88} (M=128-aligned LDWEIGHTS is ~2x
  faster than M=120), fp16 weights/activations.
"""

from contextlib import ExitStack

import numpy as np

import concourse.bass as bass
import concourse.tile as tile
import concourse.mybir as mybir
from concourse.bass_utils import run_bass_kernel_spmd

F16 = mybir.dt.float16
F32 = mybir.dt.float32
AX = mybir.AxisListType
OP = mybir.AluOpType
AF = mybir.ActivationFunctionType

B, T, DIN, D, E, H, HS = 64, 300, 768, 300, 8, 4, 16
NCORES = 8
BL = B // NCORES          # 8 batches per core
NTOK = BL * T             # 2400 tokens per core
DHK = H * HS              # 64
DH = 2 * D                # 600
TOPK = E // 2

K768 = [(i * 128, 128) for i in range(6)]
DC = [(0, 128), (128, 128), (256, 44)]          # 300 = 128+128+44
MCH = [(0, 128), (128, 128), (256, 128), (384, 128), (512, 88)]  # 600
TB = [(0, 128), (128, 128), (256, 44)]          # 300 tokens per batch
TOK = [(i * 128, 128) for i in range(18)] + [(2304, 96)]   # 2400 tokens
N5 = [(i * 480, 480) for i in range(5)]         # 2400 free-dim chunks
NEG = -1.0e30
RCLAMP = 16.0   # cap on 1/Z; keeps garbage stripe rows finite in fp16

_MAX_DRAIN_WAITS = 1
_WAIT_LIMIT = 1


def _split_waits(nc):
    """Walrus in this build caps sync waits per instruction; hoist excess
    waits onto same-engine NOPs inserted just before the instruction."""
    n = 0
    for f in nc.m.functions:
        for blk in f.blocks:
            insts = blk.instructions
            out = []
            changed = False
            for inst in insts:
                si = inst.sync_info
                waits = list(si.on_wait or []) if si is not None else []
                if len(waits) > _WAIT_LIMIT:
                    head, tail = waits[:-_WAIT_LIMIT], waits[-_WAIT_LIMIT:]
                    for i in range(0, len(head), _WAIT_LIMIT):
                        n += 1
                        nop = mybir.InstNoOp(name=f"waitnop{n}", ins=[], outs=[])
                        nop.engine = inst.engine
                        nop.sync_info = mybir.SyncInfo(
                            on_wait=head[i : i + _WAIT_LIMIT], on_update=[]
                        )
                        out.append(nop)
                    si.on_wait = tail
                    inst.sync_info = si
                    changed = True
                out.append(inst)
            if changed:
                blk.instructions = out


def _install_drain_patch():
    """This walrus build rejects CTRL instructions with more than a few sync
    waits; Tile's tail drain waits on every engine/DMA semaphore at once.
    Split the waits across a chain of single-wait drains."""
    if getattr(tile.TileContext, "_drain_patched", False):
        return

    def _patched(self, tick_clock, wait_clock):
        nc = self.nc
        drain_inst = nc.sync.drain()
        wait_clock.add_sem_waits(
            drain_inst.ins, tile.ScopedClock({None: tick_clock.global_clock})
        )
        ri = drain_inst.ins
        si = ri.sync_info
        waits = list(si.on_wait or []) if si is not None else []
        if len(waits) > _MAX_DRAIN_WAITS:
            si.on_wait = waits[:_MAX_DRAIN_WAITS]
            ri.sync_info = si
            for i in range(_MAX_DRAIN_WAITS, len(waits), _MAX_DRAIN_WAITS):
                d2 = nc.sync.drain()
                d2.ins.sync_info = mybir.SyncInfo(
                    on_wait=waits[i : i + _MAX_DRAIN_WAITS], on_update=[]
                )
        nc.all_engine_barrier()
        assert self.sems is not None
        popped = nc._tile_sem_poison_stack.pop()
        assert popped is self._sem_poison
        nc.clear_and_free_semaphores(list(self.sems.allocated().values()))
        nc.all_engine_barrier()

    tile.TileContext._drain_and_barrier = _patched
    tile.TileContext._drain_patched = True


def _build_module(b2_nonzero: bool, debug: bool = False, max_phase: int = 9):
    _install_drain_patch()
    nc = bass.Bass("TRN2", target_bir_lowering=False, debug=False)

    # ---- DRAM I/O ----
    xt_d = nc.dram_tensor("xt", [DIN, NTOK], F16, kind="ExternalInput")
    wqkv_d = nc.dram_tensor("wqkv", [DIN, 320], F16, kind="ExternalInput")
    qkvb_d = nc.dram_tensor("qkvb", [128, 3], F32, kind="ExternalInput")
    wgf_d = nc.dram_tensor("wgf", [128, E], F16, kind="ExternalInput")
    cb_d = nc.dram_tensor("cb", [E, 1], F32, kind="ExternalInput")
    w1_d = nc.dram_tensor("w1t", [E, 128, DH], F16, kind="ExternalInput")
    w2_d = nc.dram_tensor("w2t", [E, DH, D], F16, kind="ExternalInput")
    b1_d = nc.dram_tensor("b1s", [128, E * 5], F32, kind="ExternalInput")
    b2_d = nc.dram_tensor("b2s", [E, 1, D], F16, kind="ExternalInput")
    idm_d = nc.dram_tensor("idm", [128, 128], F16, kind="ExternalInput")
    out_d = nc.dram_tensor("out", [NTOK, D], F32, kind="ExternalOutput")
    if debug:
        dbg_q = nc.dram_tensor("dbg_q", [128, NTOK], F32, kind="ExternalOutput")
        dbg_k = nc.dram_tensor("dbg_k", [128, NTOK], F32, kind="ExternalOutput")
        dbg_at = nc.dram_tensor("dbg_at", [128, NTOK], F32, kind="ExternalOutput")
        dbg_xa = nc.dram_tensor("dbg_xa", [D, NTOK], F32, kind="ExternalOutput")
        dbg_gp = nc.dram_tensor("dbg_gp", [128, len(TOK) * E], F32, kind="ExternalOutput")

    with tile.TileContext(nc) as tc:
        with (
            tc.tile_pool(name="const", bufs=1) as cpool,
            tc.tile_pool(name="big", bufs=1) as bpool,
        ):
            # ---- persistent constants ----
            ident = cpool.tile([128, 128], F16)
            wqkv_s = []
            for kc, (k0, ks) in enumerate(K768):
                t = cpool.tile([128, 320], F16, tag=f"wqkv{kc}")
                nc.sync.dma_start(t[:, :], wqkv_d[k0 : k0 + ks, :])
                wqkv_s.append(t)
            qkvb_s = cpool.tile([128, 3], F32, tag="qkvbs")
            nc.sync.dma_start(qkvb_s[:, :], qkvb_d[:, :])
            wgf_s = cpool.tile([128, E], F16, tag="wgf")
            nc.sync.dma_start(wgf_s[:, :], wgf_d[:, :])
            cb_s = cpool.tile([E, 1], F32, tag="cb")
            nc.sync.dma_start(cb_s[:, :], cb_d[:, :])
            b1_s = cpool.tile([128, E * 5], F32, tag="b1s")
            nc.sync.dma_start(b1_s[:, :], b1_d[:, :])
            nc.sync.dma_start(ident[:, :], idm_d[:, :])

            # prefetch experts 0-3 weights up front on the gpsimd DMA queue
            w1_all, w2_all, b2_all = [], [], []
            for e in range(E // 2):
                t = bpool.tile([128, DH], F16, tag=f"w1_{e}", name=f"w1_{e}")
                nc.gpsimd.dma_start(t[:, :], w1_d[e, :, :])
                w1_all.append(t)
                w2t = []
                for mc, (h0, hs) in enumerate(MCH):
                    t = bpool.tile([hs, D], F16, tag=f"w2_{e}_{mc}", name=f"w2_{e}_{mc}")
                    nc.gpsimd.dma_start(t[:, :], w2_d[e, h0 : h0 + hs, :])
                    w2t.append(t)
                w2_all.append(w2t)
                if b2_nonzero:
                    t = bpool.tile([1, D], F16, tag=f"b2row{e}", name=f"b2row{e}")
                    nc.gpsimd.dma_start(t[:, :], b2_d[e, :, :])
                    b2_all.append(t)

            # ---- activations ----
            _mid = ExitStack()
            mpool = _mid.enter_context(tc.tile_pool(name="mid", bufs=1))
            # heads striped at 32-partition bases (rows 32h:32h+16 per head)
            qTs = mpool.tile([128, NTOK], F16, tag="qTs")
            kTs = mpool.tile([128, NTOK], F16, tag="kTs")
            qTb = mpool.tile([DHK, NTOK], F16, tag="qTb")
            kTb = mpool.tile([DHK, NTOK], F16, tag="kTb")
            vT = mpool.tile([DHK, NTOK], F16, tag="vT")
            # v_s[sc]: [ss, b, h, 64] = [v_h(16) | 0(16) | ones(16) | 0(16)];
            # the av/Z layout this produces keeps every vector-op partition
            # base 32-aligned
            v_s = [
                mpool.tile([128, BL, H, 64], F16, tag=f"v{sc}", name=f"v{sc}")
                for sc in range(3)
            ]
            # attnT [128, NTOK]: rows 32h:32h+16 = head h, other rows are
            # the constant 1.0 (Z/Z); folded weights are zero there
            attnT = bpool.tile([128, NTOK], F16, tag="attnT")
            # v_s constant columns [v16 | ones48]: one matmul per (h, sc)
            # produces [av (16 rows) | Z replicated (48 rows)]
            for sc in range(3):
                nc.gpsimd.memset(v_s[sc][:, :, :, 16:64], 1.0)

            # ================= phase 1: qkv =================
            with (
                tc.tile_pool(name="xt", bufs=1) as xpool,
                tc.tile_pool(name="ps1", bufs=4, space="PSUM") as ps1,
            ):
                xt_s = [
                    xpool.tile([128, NTOK], F16, tag=f"xt{kc}", name=f"xt{kc}")
                    for kc in range(6)
                ]
                for n0, ns in N5:
                    for kc, (k0, ks) in enumerate(K768):
                        nc.scalar.dma_start(
                            xt_s[kc][:, n0 : n0 + ns],
                            xt_d[k0 : k0 + ks, n0 : n0 + ns],
                        )
                specs = ((qTs, 128, 0), (kTs, 128, 128), (vT, DHK, 256))
                for n0, ns in N5:
                    for mi, (dst, mw, c0) in enumerate(specs):
                        ps = ps1.tile([128, 480], F32, tag="qkvp")
                        for kc in range(6):
                            nc.tensor.matmul(
                                ps[:mw, :ns],
                                wqkv_s[kc][:, c0 : c0 + mw],
                                xt_s[kc][:, n0 : n0 + ns],
                                start=(kc == 0), stop=(kc == 5),
                            )
                        nc.scalar.activation(
                            dst[:, n0 : n0 + ns], ps[:mw, :ns], AF.Identity,
                            bias=qkvb_s[:mw, mi : mi + 1], scale=1.0,
                        )
                # move heads 2-3 down to base 0/32 (DMA shifts partitions)
                nc.sync.dma_start(qTb[:, :], qTs[64:128, :])
                nc.sync.dma_start(kTb[:, :], kTs[64:128, :])
                if debug:
                    dq = mpool.tile([128, NTOK], F32, tag="dbgq")
                    nc.vector.tensor_copy(dq[:, :], qTs[:, :])
                    nc.sync.dma_start(dbg_q[:, :], dq[:, :])
                    dk = mpool.tile([128, NTOK], F32, tag="dbgk")
                    nc.vector.tensor_copy(dk[:, :], kTs[:, :])
                    nc.sync.dma_start(dbg_k[:, :], dk[:, :])

            # ================= phase 2: attention =================
            if max_phase >= 2:
             with (
                tc.tile_pool(name="att", bufs=2) as apool,
                tc.tile_pool(name="ps2", bufs=2, space="PSUM") as ps2,
                tc.tile_pool(name="ps2z", bufs=2, space="PSUM") as ps2z,
            ):
                for b in range(BL):
                    base = b * T
                    # v = vT.T per s-chunk (all heads at once)
                    for sc, (s0, ss) in enumerate(TB):
                        vtr = ps2.tile([128, DHK], F16, tag="vtr")
                        nc.tensor.transpose(
                            vtr[:ss, :], vT[:, base + s0 : base + s0 + ss],
                            ident[0:DHK, 0:DHK],
                        )
                        nc.scalar.copy(
                            v_s[sc][0:ss, b, :, 0:16],
                            vtr[:ss, :].rearrange("p (h k) -> p h k", h=H),
                        )
                    # FT pass: scores [s, t] with the causal -30000 mask added
                    # into the diag block via an identity-stationary matmul;
                    # exp; then one matmul per (h, sc) computing
                    # [av_h | 0 | Z_h | 0] in a 64-row block; heads 2g / 2g+1
                    # at row bases 0 / 64 of azb[g]
                    azb = [
                        ps2z.tile([128, T], F32, tag=f"azb{g}", name=f"azb{g}")
                        for g in range(2)
                    ]
                    ft = {}
                    for h in range(H):
                        qt = qTs if h < 2 else qTb
                        kt = kTs if h < 2 else kTb
                        q0 = k0 = (h % 2) * 32
                        for sc, (s0, ss) in enumerate(TB):
                            tlen = T - s0
                            weiT = ps2.tile([128, T], F32, tag="weiT")
                            nc.tensor.matmul(
                                weiT[:ss, :tlen],
                                kt[k0 : k0 + HS, base + s0 : base + s0 + ss],
                                qt[q0 : q0 + HS, base + s0 : base + T],
                                start=True, stop=True,
                            )
                            f = apool.tile([128, T], F16, tag=f"ft{h}{sc}")
                            ft[(h, sc)] = f
                            nc.scalar.activation(f[:ss, :tlen], weiT[:ss, :tlen], AF.Exp)
                            nc.gpsimd.affine_select(
                                out=f[:ss, :ss], in_=f[:ss, :ss],
                                compare_op=OP.is_ge, fill=0.0,
                                base=0, channel_multiplier=-1, pattern=[[1, ss]],
                            )
                        for sc, (s0, ss) in enumerate(TB):
                            nc.tensor.matmul(
                                azb[h // 2][64 * (h % 2) : 64 * (h % 2) + 64, s0:T],
                                v_s[sc][0:ss, b, h, :],
                                ft[(h, sc)][0:ss, 0 : T - s0],
                                start=(sc == 0), stop=(sc == 2),
                            )
                    # 1/azb on the scalar table engine (cheap); only the
                    # Z-recip rows are ever read back, so the garbage
                    # av-recip rows (possibly inf) are harmless
                    rz = [
                        apool.tile([128, T], F32, tag=f"rz{g}", name=f"rz{g}")
                        for g in range(2)
                    ]
                    for g in range(2):
                        nc.vector.reciprocal(rz[g][:, :], azb[g][:, :])
                    for h in range(H):
                        g, o = h // 2, 64 * (h % 2)
                        nc.vector.scalar_tensor_tensor(
                            attnT[32 * h : 32 * h + 32, base : base + T],
                            rz[g][o + 32 : o + 64, :], RCLAMP,
                            azb[g][o : o + 32, :],
                            op0=OP.min, op1=OP.mult,
                        )
                if debug:
                    da = apool.tile([128, NTOK], F32, tag="dbga")
                    nc.vector.tensor_copy(da[:, :], attnT[:, :])
                    nc.sync.dma_start(dbg_at[:, :], da[:, :])

            _mid.close()
            _late = ExitStack()
            lpool = _late.enter_context(tc.tile_pool(name="late", bufs=1))
            gp = lpool.tile([128, len(TOK) * E], F32, tag="gp")
            G = lpool.tile([128, len(TOK) * D], F32, tag="G")
            S = lpool.tile([128, len(TOK) * D], F32, tag="S")

            # ================= phase 4: gating =================
            if max_phase >= 4:
             with (
                tc.tile_pool(name="gat", bufs=1) as gpool,
                tc.tile_pool(name="ps4", bufs=2, space="PSUM") as ps4,
                tc.tile_pool(name="ps4t", bufs=1, space="PSUM") as ps4t,
             ):
                NT_ = len(TOK)
                # logits transposed: [8, tok], folded proj+gate stationary
                logT = gpool.tile([8, NTOK], F16, tag="logT")
                for n0, ns in N5:
                    lg = ps4.tile([8, 480], F32, tag="lg")
                    nc.tensor.matmul(
                        lg[:, :ns], wgf_s[:, :], attnT[:, n0 : n0 + ns],
                        start=True, stop=True,
                    )
                    nc.scalar.activation(
                        logT[:, n0 : n0 + ns], lg[:, :ns], AF.Identity,
                        bias=cb_s[:, :], scale=1.0,
                    )
                # transpose back to [tok, 8] in one psum bank
                ltp = ps4t.tile([128, NT_, E], F16, tag="ltp")
                for ti, (t0, ts_) in enumerate(TOK):
                    nc.tensor.transpose(
                        ltp[:ts_, ti, :], logT[:, t0 : t0 + ts_], ident[0:8, 0:8]
                    )
                L3 = gpool.tile([128, NT_, E], F32, tag="L3")
                nc.scalar.copy(L3[:, :, :], ltp[:, :, :])
                work = gpool.tile([128, NT_, E], F32, tag="work")
                nc.vector.tensor_copy(work[:, :, :], L3[:, :, :])
                m = gpool.tile([128, NT_, 4], F32, tag="m")
                eqm = gpool.tile([128, NT_, E], F32, tag="eqm")
                for r in range(TOPK):
                    nc.vector.tensor_reduce(
                        m[:, :, r : r + 1], work[:, :, :], axis=AX.X, op=OP.max,
                        opt_input=False, opt_output=False,
                    )
                    if r < TOPK - 1:
                        nc.vector.tensor_tensor(
                            eqm[:, :, :], work[:, :, :],
                            m[:, :, r : r + 1].broadcast_to((128, NT_, E)),
                            op=OP.is_equal,
                        )
                        nc.vector.scalar_tensor_tensor(
                            work[:, :, :], eqm[:, :, :], NEG, work[:, :, :],
                            op0=OP.mult, op1=OP.add,
                        )
                sel = gpool.tile([128, NT_, E], F32, tag="sel")
                nc.vector.tensor_tensor(
                    sel[:, :, :], L3[:, :, :],
                    m[:, :, 3:4].broadcast_to((128, NT_, E)), op=OP.is_ge,
                )
                el = gpool.tile([128, NT_, E], F32, tag="el")
                nc.scalar.activation(el[:, :, :], L3[:, :, :], AF.Exp)
                elm = gpool.tile([128, NT_, E], F32, tag="elm")
                nc.vector.tensor_tensor(elm[:, :, :], el[:, :, :], sel[:, :, :], op=OP.mult)
                zg = gpool.tile([128, NT_, 1], F32, tag="zg")
                nc.vector.tensor_reduce(
                    zg[:, :, :], elm[:, :, :], axis=AX.X, op=OP.add,
                    opt_input=False, opt_output=False,
                )
                rzg = gpool.tile([128, NT_, 1], F32, tag="rzg")
                nc.vector.reciprocal(rzg[:, :, :], zg[:, :, :])
                nc.vector.tensor_tensor(
                    gp[:, :].rearrange("p (t e) -> p t e", e=E), elm[:, :, :],
                    rzg[:, :, :].broadcast_to((128, NT_, E)),
                    op=OP.mult,
                )
            if debug and max_phase >= 4:
                nc.sync.dma_start(dbg_gp[:, :], gp[:, :])

            # ================= phase 5: experts =================
            if max_phase >= 5:
             with (
                tc.tile_pool(name="outp", bufs=4) as opool,
                tc.tile_pool(name="wexp", bufs=1) as wpool,
                tc.tile_pool(name="ht", bufs=2) as hpool,
                tc.tile_pool(name="ps5", bufs=4, space="PSUM") as ps5h,
                tc.tile_pool(name="ps5b", bufs=4, space="PSUM") as ps5e,
            ):
                for e in range(E // 2, E):
                    t = wpool.tile([128, DH], F16, tag=f"w1_{e}", name=f"w1_{e}")
                    nc.gpsimd.dma_start(t[:, :], w1_d[e, :, :])
                    w1_all.append(t)
                    w2t = []
                    for mc, (h0, hs) in enumerate(MCH):
                        t = wpool.tile([hs, D], F16, tag=f"w2_{e}_{mc}", name=f"w2_{e}_{mc}")
                        nc.gpsimd.dma_start(t[:, :], w2_d[e, h0 : h0 + hs, :])
                        w2t.append(t)
                    w2_all.append(w2t)
                    if b2_nonzero:
                        t = wpool.tile([1, D], F16, tag=f"b2row{e}", name=f"b2row{e}")
                        nc.gpsimd.dma_start(t[:, :], b2_d[e, :, :])
                        b2_all.append(t)
                ones = None
                if b2_nonzero:
                    ones = cpool.tile([1, NTOK], F16, tag="ones")
                    nc.gpsimd.memset(ones[:, :], 1.0)
                for e in range(E):
                    w1t = w1_all[e]
                    w2t = w2_all[e]
                    b2row = b2_all[e] if b2_nonzero else None
                    ht = []
                    for mc, (h0, hs) in enumerate(MCH):
                        t = hpool.tile([hs, NTOK], F16, tag=f"ht{mc}")
                        ht.append(t)
                        for n, (n0, ns) in enumerate(N5):
                            hp = ps5h.tile([128, 480], F32, tag="hp")
                            nc.tensor.matmul(
                                hp[:hs, :ns], w1t[:, h0 : h0 + hs],
                                attnT[:, n0 : n0 + ns],
                                start=True, stop=True,
                            )
                            nc.scalar.activation(
                                t[:, n0 : n0 + ns], hp[:hs, :ns], AF.Relu,
                                bias=b1_s[:hs, e * 5 + mc : e * 5 + mc + 1], scale=1.0,
                            )
                    for ti, (t0, ts_) in enumerate(TOK):
                        eo = ps5e.tile([128, D], F32, tag="eo")
                        for mc in range(5):
                            nc.tensor.matmul(
                                eo[:ts_, :], ht[mc][:, t0 : t0 + ts_], w2t[mc][:, :],
                                start=(mc == 0), stop=(mc == 4 and not b2_nonzero),
                            )
                        if b2_nonzero:
                            nc.tensor.matmul(
                                eo[:ts_, :], ones[:, t0 : t0 + ts_], b2row[:, :],
                                start=False, stop=True,
                            )
                        gsc = gp[:ts_, ti * E + e : ti * E + e + 1]
                        gsl = G[:ts_, ti * D : (ti + 1) * D]
                        ssl = S[:ts_, ti * D : (ti + 1) * D]
                        if e == 0:
                            nc.vector.tensor_scalar_mul(gsl, eo[:ts_, :], gsc)
                            nc.scalar.copy(ssl, eo[:ts_, :])
                        else:
                            nc.vector.scalar_tensor_tensor(
                                gsl, eo[:ts_, :], gsc, gsl, op0=OP.mult, op1=OP.add
                            )
                            nc.vector.tensor_tensor(ssl, eo[:ts_, :], ssl, op=OP.add)
                        if e == E - 1:
                            o = opool.tile([128, D], F32, tag="o")
                            nc.vector.tensor_tensor(o[:ts_, :], gsl, ssl, op=OP.mult)
                            nc.sync.dma_start(out_d[t0 : t0 + ts_, :], o[:ts_, :])

            _late.close()

    _split_waits(nc)
    return nc


_CACHE = {}
LAST_RESULT = None


def _get_module(b2_nonzero: bool, debug: bool = False, max_phase: int = 9):
    key = (b2_nonzero, debug, max_phase)
    if key not in _CACHE:
        _CACHE[key] = _build_module(b2_nonzero, debug=debug, max_phase=max_phase)
    return _CACHE[key]


def _prep_inputs(x, wh_bias, wh_W, Wq, Wk, Wv, proj_W, proj_b,
                 exp_W1, exp_b1, exp_W2, exp_b2, w_gate):
    # fold whiten into qkv; fold attention scale into q
    scale = float(D) ** -0.5
    Wqf = (Wq.reshape(DHK, D) @ wh_W) * scale          # [64, 768]
    Wkf = Wk.reshape(DHK, D) @ wh_W
    Wvf = Wv.reshape(DHK, D) @ wh_W

    def stripe(w):                                     # [64, 768] -> [128, 768]
        out = np.zeros((128, DIN), np.float32)
        for h in range(H):
            out[h * 32 : h * 32 + HS] = w[h * HS : (h + 1) * HS]
        return out

    def stripe_b(v):                                   # [64] -> [128]
        out = np.zeros(128, np.float32)
        for h in range(H):
            out[h * 32 : h * 32 + HS] = v[h * HS : (h + 1) * HS]
        return out

    wqkv = np.concatenate([stripe(Wqf), stripe(Wkf), Wvf], 0)   # [320, 768]
    bq = -(Wqf @ wh_bias)
    bk = -(Wkf @ wh_bias)
    bv = -(Wvf @ wh_bias)
    qkvb = np.stack([stripe_b(bq), stripe_b(bk), np.pad(bv, (0, 64))], 1)  # [128, 3]

    # head-striped proj weights [128, D]: rows 32h:32h+16 = head h, zeros
    # elsewhere (attnT garbage rows are the constant Z/Z = 1 there)
    projwS = np.zeros((128, D), np.float64)
    for h in range(H):
        projwS[32 * h : 32 * h + HS] = proj_W[:, h * HS : (h + 1) * HS].T

    # fold proj into the gate and expert W1 weights (fp64 host math)
    wgf = projwS @ w_gate.astype(np.float64)               # [128, E]
    cb = proj_b.astype(np.float64) @ w_gate.astype(np.float64)   # [E]
    w1f = np.einsum(
        "pd,ehd->eph", projwS, exp_W1.astype(np.float64)
    )                                                      # [E, 128, 600]
    b1f = exp_W1.astype(np.float64) @ proj_b.astype(np.float64) + exp_b1  # [E, 600]

    # b1 bias slices [128, E*5]: column e*5+mc holds b1f[e, h0:h0+hs]
    b1s = np.zeros((128, E * 5), np.float32)
    for e in range(E):
        for mc, (h0, hs) in enumerate(MCH):
            b1s[:hs, e * 5 + mc] = b1f[e, h0 : h0 + hs]

    f16 = np.float16
    common = {
        "wqkv": np.ascontiguousarray(wqkv.T).astype(f16),
        "qkvb": np.ascontiguousarray(qkvb).astype(np.float32),
        "wgf": np.ascontiguousarray(wgf).astype(f16),
        "cb": np.ascontiguousarray(cb[:, None]).astype(np.float32),
        "idm": np.eye(128, dtype=f16),
        "w1t": np.ascontiguousarray(w1f).astype(f16),
        "w2t": np.ascontiguousarray(exp_W2.transpose(0, 2, 1)).astype(f16),
        "b1s": b1s.astype(np.float32),
        "b2s": np.ascontiguousarray(exp_b2[:, None, :]).astype(f16),
    }
    in_maps = []
    for c in range(NCORES):
        xc = x[c * BL : (c + 1) * BL]                  # [8, 300, 768]
        xt = np.ascontiguousarray(
            xc.transpose(2, 0, 1).reshape(DIN, NTOK)
        ).astype(f16)
        in_maps.append({**common, "xt": xt})
    return in_maps


def kernel(x, wh_bias, wh_W, Wq, Wk, Wv, proj_W, proj_b,
           exp_W1, exp_b1, exp_W2, exp_b2, w_gate,
           debug=False, max_phase=9):
    global LAST_RESULT
    x = np.asarray(x, np.float32)
    wh_bias = np.asarray(wh_bias, np.float32)
    wh_W = np.asarray(wh_W, np.float32)
    Wq, Wk, Wv = (np.asarray(w, np.float32) for w in (Wq, Wk, Wv))
    proj_W = np.asarray(proj_W, np.float32)
    proj_b = np.asarray(proj_b, np.float32)
    exp_W1 = np.asarray(exp_W1, np.float32)
    exp_b1 = np.asarray(exp_b1, np.float32)
    exp_W2 = np.asarray(exp_W2, np.float32)
    exp_b2 = np.asarray(exp_b2, np.float32)
    w_gate = np.asarray(w_gate, np.float32)

    b2_nonzero = bool(np.any(exp_b2))
    in_maps = _prep_inputs(x, wh_bias, wh_W, Wq, Wk, Wv, proj_W, proj_b,
                           exp_W1, exp_b1, exp_W2, exp_b2, w_gate)

    nc = _get_module(b2_nonzero, debug=debug, max_phase=max_phase)
    for alloc in nc.m.functions[0].allocations:
        if isinstance(alloc, mybir.MemoryLocationSet) and alloc.kind == "ExternalInput":
            nm = alloc.memorylocations[0].name
            if nm not in in_maps[0]:
                continue  # partition_id etc., supplied by the runner
            got = in_maps[0][nm]
            assert tuple(got.shape) == tuple(alloc.tensor_shape), (
                nm, got.shape, alloc.tensor_shape)
            assert got.dtype == mybir.dt.np(alloc.dtype), (nm, got.dtype)
    res = run_bass_kernel_spmd(nc, in_maps, core_ids=list(range(NCORES)))
    LAST_RESULT = res
    out = np.stack([r["out"] for r in res.results])    # [8, 2400, 300]
    return out.reshape(B, T, D)


# revision 33
# speedup vs baseline: 1.0107x; 1.0107x over previous
"""Trainium2 Bass kernel for nn_MoEAdaptorLayer (whiten -> causal MHA -> proj
-> noisy-top-k gating (eval) -> 8 dense experts -> gated mixture * expert sum).

Sharding: data-parallel over batch. 64 batches -> 8 per core, params replicated.

v2 redesign vs baseline:
- fp16 everywhere on the matmul path (fp32 PSUM accumulation); halves DMA and
  removes the fp32r small-free-dim PE penalties.
- qkv: x shipped as one [768, 2400] fp16 tensor per core (6 big DMAs), matmuls
  in [128, 480] chunks.
- attention: single-orientation flash-style pass. Scores computed [s, t] only;
  exp on scalar; causal mask by multiplying the diagonal blocks with a
  triangular constant (split vector/gpsimd). AV and the softmax denominator
  come from ONE matmul per (head, s-chunk): stationary [v_h | ones] (32 cols)
  so the psum holds [av_h (16 rows) | Z_h replicated (16 rows)] per 32-stripe.
  One reciprocal + one fused min-mult STT per batch produce attnT directly in
  head-striped [128, tok] layout (no Z-pass, no output transposes).
- proj: stationary is zero-padded to the 32-stripe layout; 15 fp16 matmuls.
- gating: logits computed transposed ([8, tok], weight-stationary, 15 matmuls
  instead of 57 LDW-bound ones), PE-transposed back in 19 tiny transposes.
- experts: h chunked {128,128,128,128,88} (M=128-aligned LDWEIGHTS is ~2x
  faster than M=120), fp16 weights/activations.
"""

from contextlib import ExitStack

import numpy as np

import concourse.bass as bass
import concourse.tile as tile
import concourse.mybir as mybir
from concourse.bass_utils import run_bass_kernel_spmd

F16 = mybir.dt.float16
F32 = mybir.dt.float32
AX = mybir.AxisListType
OP = mybir.AluOpType
AF = mybir.ActivationFunctionType

B, T, DIN, D, E, H, HS = 64, 300, 768, 300, 8, 4, 16
NCORES = 8
BL = B // NCORES          # 8 batches per core
NTOK = BL * T             # 2400 tokens per core
DHK = H * HS              # 64
DH = 2 * D                # 600
TOPK = E // 2

K768 = [(i * 128, 128) for i in range(6)]
DC = [(0, 128), (128, 128), (256, 44)]          # 300 = 128+128+44
MCH = [(0, 128), (128, 128), (256, 128), (384, 128), (512, 88)]  # 600
TB = [(0, 128), (128, 128), (256, 44)]          # 300 tokens per batch
TOK = [(i * 128, 128) for i in range(18)] + [(2304, 96)]   # 2400 tokens
N5 = [(i * 480, 480) for i in range(5)]         # 2400 free-dim chunks
NEG = -1.0e30
RCLAMP = 16.0   # cap on 1/Z; keeps garbage stripe rows finite in fp16

_MAX_DRAIN_WAITS = 1
_WAIT_LIMIT = 1


def _split_waits(nc):
    """Walrus in this build caps sync waits per instruction; hoist excess
    waits onto same-engine NOPs inserted just before the instruction."""
    n = 0
    for f in nc.m.functions:
        for blk in f.blocks:
            insts = blk.instructions
            out = []
            changed = False
            for inst in insts:
                si = inst.sync_info
                waits = list(si.on_wait or []) if si is not None else []
                if len(waits) > _WAIT_LIMIT:
                    head, tail = waits[:-_WAIT_LIMIT], waits[-_WAIT_LIMIT:]
                    for i in range(0, len(head), _WAIT_LIMIT):
                        n += 1
                        nop = mybir.InstNoOp(name=f"waitnop{n}", ins=[], outs=[])
                        nop.engine = inst.engine
                        nop.sync_info = mybir.SyncInfo(
                            on_wait=head[i : i + _WAIT_LIMIT], on_update=[]
                        )
                        out.append(nop)
                    si.on_wait = tail
                    inst.sync_info = si
                    changed = True
                out.append(inst)
            if changed:
                blk.instructions = out


def _install_drain_patch():
    """This walrus build rejects CTRL instructions with more than a few sync
    waits; Tile's tail drain waits on every engine/DMA semaphore at once.
    Split the waits across a chain of single-wait drains."""
    if getattr(tile.TileContext, "_drain_patched", False):
        return

    def _patched(self, tick_clock, wait_clock):
        nc = self.nc
        drain_inst = nc.sync.drain()
        wait_clock.add_sem_waits(
            drain_inst.ins, tile.ScopedClock({None: tick_clock.global_clock})
        )
        ri = drain_inst.ins
        si = ri.sync_info
        waits = list(si.on_wait or []) if si is not None else []
        if len(waits) > _MAX_DRAIN_WAITS:
            si.on_wait = waits[:_MAX_DRAIN_WAITS]
            ri.sync_info = si
            for i in range(_MAX_DRAIN_WAITS, len(waits), _MAX_DRAIN_WAITS):
                d2 = nc.sync.drain()
                d2.ins.sync_info = mybir.SyncInfo(
                    on_wait=waits[i : i + _MAX_DRAIN_WAITS], on_update=[]
                )
        nc.all_engine_barrier()
        assert self.sems is not None
        popped = nc._tile_sem_poison_stack.pop()
        assert popped is self._sem_poison
        nc.clear_and_free_semaphores(list(self.sems.allocated().values()))
        nc.all_engine_barrier()

    tile.TileContext._drain_and_barrier = _patched
    tile.TileContext._drain_patched = True


def _build_module(b2_nonzero: bool, debug: bool = False, max_phase: int = 9):
    _install_drain_patch()
    nc = bass.Bass("TRN2", target_bir_lowering=False, debug=False)

    # ---- DRAM I/O ----
    xt_d = nc.dram_tensor("xt", [DIN, NTOK], F16, kind="ExternalInput")
    wqkv_d = nc.dram_tensor("wqkv", [DIN, 320], F16, kind="ExternalInput")
    qkvb_d = nc.dram_tensor("qkvb", [128, 3], F32, kind="ExternalInput")
    wgf_d = nc.dram_tensor("wgf", [128, E], F16, kind="ExternalInput")
    cb_d = nc.dram_tensor("cb", [E, 1], F32, kind="ExternalInput")
    w1_d = nc.dram_tensor("w1t", [E, 128, DH], F16, kind="ExternalInput")
    w2_d = nc.dram_tensor("w2t", [E, DH, D], F16, kind="ExternalInput")
    b1_d = nc.dram_tensor("b1s", [128, E * 5], F32, kind="ExternalInput")
    b2_d = nc.dram_tensor("b2s", [E, 1, D], F16, kind="ExternalInput")
    idm_d = nc.dram_tensor("idm", [128, 128], F16, kind="ExternalInput")
    out_d = nc.dram_tensor("out", [NTOK, D], F32, kind="ExternalOutput")
    if debug:
        dbg_q = nc.dram_tensor("dbg_q", [128, NTOK], F32, kind="ExternalOutput")
        dbg_k = nc.dram_tensor("dbg_k", [128, NTOK], F32, kind="ExternalOutput")
        dbg_at = nc.dram_tensor("dbg_at", [128, NTOK], F32, kind="ExternalOutput")
        dbg_xa = nc.dram_tensor("dbg_xa", [D, NTOK], F32, kind="ExternalOutput")
        dbg_gp = nc.dram_tensor("dbg_gp", [128, len(TOK) * E], F32, kind="ExternalOutput")

    with tile.TileContext(nc) as tc:
        with (
            tc.tile_pool(name="const", bufs=1) as cpool,
            tc.tile_pool(name="big", bufs=1) as bpool,
        ):
            # ---- persistent constants ----
            ident = cpool.tile([128, 128], F16)
            wqkv_s = []
            for kc, (k0, ks) in enumerate(K768):
                t = cpool.tile([128, 320], F16, tag=f"wqkv{kc}")
                nc.sync.dma_start(t[:, :], wqkv_d[k0 : k0 + ks, :])
                wqkv_s.append(t)
            qkvb_s = cpool.tile([128, 3], F32, tag="qkvbs")
            nc.sync.dma_start(qkvb_s[:, :], qkvb_d[:, :])
            wgf_s = cpool.tile([128, E], F16, tag="wgf")
            nc.sync.dma_start(wgf_s[:, :], wgf_d[:, :])
            cb_s = cpool.tile([E, 1], F32, tag="cb")
            nc.sync.dma_start(cb_s[:, :], cb_d[:, :])
            b1_s = cpool.tile([128, E * 5], F32, tag="b1s")
            nc.sync.dma_start(b1_s[:, :], b1_d[:, :])
            nc.sync.dma_start(ident[:, :], idm_d[:, :])

            # prefetch experts 0-3 weights up front on the gpsimd DMA queue
            w1_all, w2_all, b2_all = [], [], []
            for e in range(E // 2):
                t = bpool.tile([128, DH], F16, tag=f"w1_{e}", name=f"w1_{e}")
                nc.gpsimd.dma_start(t[:, :], w1_d[e, :, :])
                w1_all.append(t)
                w2t = []
                for mc, (h0, hs) in enumerate(MCH):
                    t = bpool.tile([hs, D], F16, tag=f"w2_{e}_{mc}", name=f"w2_{e}_{mc}")
                    nc.gpsimd.dma_start(t[:, :], w2_d[e, h0 : h0 + hs, :])
                    w2t.append(t)
                w2_all.append(w2t)
                if b2_nonzero:
                    t = bpool.tile([1, D], F16, tag=f"b2row{e}", name=f"b2row{e}")
                    nc.gpsimd.dma_start(t[:, :], b2_d[e, :, :])
                    b2_all.append(t)

            # ---- activations ----
            _mid = ExitStack()
            mpool = _mid.enter_context(tc.tile_pool(name="mid", bufs=1))
            # heads striped at 32-partition bases (rows 32h:32h+16 per head)
            qTs = mpool.tile([128, NTOK], F16, tag="qTs")
            kTs = mpool.tile([128, NTOK], F16, tag="kTs")
            qTb = mpool.tile([DHK, NTOK], F16, tag="qTb")
            kTb = mpool.tile([DHK, NTOK], F16, tag="kTb")
            vT = mpool.tile([DHK, NTOK], F16, tag="vT")
            # v_s[sc]: [ss, b, h, 64] = [v_h(16) | 0(16) | ones(16) | 0(16)];
            # the av/Z layout this produces keeps every vector-op partition
            # base 32-aligned
            v_s = [
                mpool.tile([128, BL, H, 64], F16, tag=f"v{sc}", name=f"v{sc}")
                for sc in range(3)
            ]
            # attnT [128, NTOK]: rows 32h:32h+16 = head h, other rows are
            # the constant 1.0 (Z/Z); folded weights are zero there
            attnT = bpool.tile([128, NTOK], F16, tag="attnT")
            # v_s constant columns [v16 | ones48]: one matmul per (h, sc)
            # produces [av (16 rows) | Z replicated (48 rows)]
            for sc in range(3):
                nc.gpsimd.memset(v_s[sc][:, :, :, 16:64], 1.0)

            # ================= phase 1: qkv =================
            with (
                tc.tile_pool(name="xt", bufs=1) as xpool,
                tc.tile_pool(name="ps1", bufs=4, space="PSUM") as ps1,
            ):
                xt_s = [
                    xpool.tile([128, NTOK], F16, tag=f"xt{kc}", name=f"xt{kc}")
                    for kc in range(6)
                ]
                for kc, (k0, ks) in enumerate(K768):
                    nc.scalar.dma_start(
                        xt_s[kc][:, 0:480], xt_d[k0 : k0 + ks, 0:480]
                    )
                for kc, (k0, ks) in enumerate(K768):
                    nc.scalar.dma_start(
                        xt_s[kc][:, 480:NTOK], xt_d[k0 : k0 + ks, 480:NTOK]
                    )
                specs = ((qTs, 128, 0), (kTs, 128, 128), (vT, DHK, 256))
                for n0, ns in N5:
                    for mi, (dst, mw, c0) in enumerate(specs):
                        ps = ps1.tile([128, 480], F32, tag="qkvp")
                        for kc in range(6):
                            nc.tensor.matmul(
                                ps[:mw, :ns],
                                wqkv_s[kc][:, c0 : c0 + mw],
                                xt_s[kc][:, n0 : n0 + ns],
                                start=(kc == 0), stop=(kc == 5),
                            )
                        nc.scalar.activation(
                            dst[:, n0 : n0 + ns], ps[:mw, :ns], AF.Identity,
                            bias=qkvb_s[:mw, mi : mi + 1], scale=1.0,
                        )
                # move heads 2-3 down to base 0/32 (DMA shifts partitions)
                nc.sync.dma_start(qTb[:, :], qTs[64:128, :])
                nc.sync.dma_start(kTb[:, :], kTs[64:128, :])
                if debug:
                    dq = mpool.tile([128, NTOK], F32, tag="dbgq")
                    nc.vector.tensor_copy(dq[:, :], qTs[:, :])
                    nc.sync.dma_start(dbg_q[:, :], dq[:, :])
                    dk = mpool.tile([128, NTOK], F32, tag="dbgk")
                    nc.vector.tensor_copy(dk[:, :], kTs[:, :])
                    nc.sync.dma_start(dbg_k[:, :], dk[:, :])

            # ================= phase 2: attention =================
            if max_phase >= 2:
             with (
                tc.tile_pool(name="att", bufs=2) as apool,
                tc.tile_pool(name="ps2", bufs=2, space="PSUM") as ps2,
                tc.tile_pool(name="ps2z", bufs=2, space="PSUM") as ps2z,
            ):
                for b in range(BL):
                    base = b * T
                    # v = vT.T per s-chunk (all heads at once)
                    for sc, (s0, ss) in enumerate(TB):
                        vtr = ps2.tile([128, DHK], F16, tag="vtr")
                        nc.tensor.transpose(
                            vtr[:ss, :], vT[:, base + s0 : base + s0 + ss],
                            ident[0:DHK, 0:DHK],
                        )
                        nc.scalar.copy(
                            v_s[sc][0:ss, b, :, 0:16],
                            vtr[:ss, :].rearrange("p (h k) -> p h k", h=H),
                        )
                    # FT pass: scores [s, t] with the causal -30000 mask added
                    # into the diag block via an identity-stationary matmul;
                    # exp; then one matmul per (h, sc) computing
                    # [av_h | 0 | Z_h | 0] in a 64-row block; heads 2g / 2g+1
                    # at row bases 0 / 64 of azb[g]
                    azb = [
                        ps2z.tile([128, T], F32, tag=f"azb{g}", name=f"azb{g}")
                        for g in range(2)
                    ]
                    ft = {}
                    for h in range(H):
                        qt = qTs if h < 2 else qTb
                        kt = kTs if h < 2 else kTb
                        q0 = k0 = (h % 2) * 32
                        for sc, (s0, ss) in enumerate(TB):
                            tlen = T - s0
                            weiT = ps2.tile([128, T], F32, tag="weiT")
                            nc.tensor.matmul(
                                weiT[:ss, :tlen],
                                kt[k0 : k0 + HS, base + s0 : base + s0 + ss],
                                qt[q0 : q0 + HS, base + s0 : base + T],
                                start=True, stop=True,
                            )
                            f = apool.tile([128, T], F16, tag=f"ft{h}{sc}")
                            ft[(h, sc)] = f
                            nc.scalar.activation(f[:ss, :tlen], weiT[:ss, :tlen], AF.Exp)
                            nc.gpsimd.affine_select(
                                out=f[:ss, :ss], in_=f[:ss, :ss],
                                compare_op=OP.is_ge, fill=0.0,
                                base=0, channel_multiplier=-1, pattern=[[1, ss]],
                            )
                        for sc, (s0, ss) in enumerate(TB):
                            nc.tensor.matmul(
                                azb[h // 2][64 * (h % 2) : 64 * (h % 2) + 64, s0:T],
                                v_s[sc][0:ss, b, h, :],
                                ft[(h, sc)][0:ss, 0 : T - s0],
                                start=(sc == 0), stop=(sc == 2),
                            )
                    # 1/azb on the scalar table engine (cheap); only the
                    # Z-recip rows are ever read back, so the garbage
                    # av-recip rows (possibly inf) are harmless
                    rz = [
                        apool.tile([128, T], F32, tag=f"rz{g}", name=f"rz{g}")
                        for g in range(2)
                    ]
                    for g in range(2):
                        nc.vector.reciprocal(rz[g][:, :], azb[g][:, :])
                    for h in range(H):
                        g, o = h // 2, 64 * (h % 2)
                        nc.vector.scalar_tensor_tensor(
                            attnT[32 * h : 32 * h + 32, base : base + T],
                            rz[g][o + 32 : o + 64, :], RCLAMP,
                            azb[g][o : o + 32, :],
                            op0=OP.min, op1=OP.mult,
                        )
                if debug:
                    da = apool.tile([128, NTOK], F32, tag="dbga")
                    nc.vector.tensor_copy(da[:, :], attnT[:, :])
                    nc.sync.dma_start(dbg_at[:, :], da[:, :])

            _mid.close()
            _late = ExitStack()
            lpool = _late.enter_context(tc.tile_pool(name="late", bufs=1))
            gp = lpool.tile([128, len(TOK) * E], F32, tag="gp")
            G = lpool.tile([128, len(TOK) * D], F32, tag="G")
            S = lpool.tile([128, len(TOK) * D], F32, tag="S")

            # ================= phase 4: gating =================
            if max_phase >= 4:
             with (
                tc.tile_pool(name="gat", bufs=1) as gpool,
                tc.tile_pool(name="ps4", bufs=2, space="PSUM") as ps4,
                tc.tile_pool(name="ps4t", bufs=1, space="PSUM") as ps4t,
             ):
                NT_ = len(TOK)
                # logits transposed: [8, tok], folded proj+gate stationary
                logT = gpool.tile([8, NTOK], F16, tag="logT")
                for n0, ns in N5:
                    lg = ps4.tile([8, 480], F32, tag="lg")
                    nc.tensor.matmul(
                        lg[:, :ns], wgf_s[:, :], attnT[:, n0 : n0 + ns],
                        start=True, stop=True,
                    )
                    nc.scalar.activation(
                        logT[:, n0 : n0 + ns], lg[:, :ns], AF.Identity,
                        bias=cb_s[:, :], scale=1.0,
                    )
                # transpose back to [tok, 8] in one psum bank
                ltp = ps4t.tile([128, NT_, E], F16, tag="ltp")
                for ti, (t0, ts_) in enumerate(TOK):
                    nc.tensor.transpose(
                        ltp[:ts_, ti, :], logT[:, t0 : t0 + ts_], ident[0:8, 0:8]
                    )
                L3 = gpool.tile([128, NT_, E], F32, tag="L3")
                nc.scalar.copy(L3[:, :, :], ltp[:, :, :])
                work = gpool.tile([128, NT_, E], F32, tag="work")
                nc.vector.tensor_copy(work[:, :, :], L3[:, :, :])
                m = gpool.tile([128, NT_, 4], F32, tag="m")
                eqm = gpool.tile([128, NT_, E], F32, tag="eqm")
                for r in range(TOPK):
                    nc.vector.tensor_reduce(
                        m[:, :, r : r + 1], work[:, :, :], axis=AX.X, op=OP.max,
                        opt_input=False, opt_output=False,
                    )
                    if r < TOPK - 1:
                        nc.vector.tensor_tensor(
                            eqm[:, :, :], work[:, :, :],
                            m[:, :, r : r + 1].broadcast_to((128, NT_, E)),
                            op=OP.is_equal,
                        )
                        nc.vector.scalar_tensor_tensor(
                            work[:, :, :], eqm[:, :, :], NEG, work[:, :, :],
                            op0=OP.mult, op1=OP.add,
                        )
                sel = gpool.tile([128, NT_, E], F32, tag="sel")
                nc.vector.tensor_tensor(
                    sel[:, :, :], L3[:, :, :],
                    m[:, :, 3:4].broadcast_to((128, NT_, E)), op=OP.is_ge,
                )
                el = gpool.tile([128, NT_, E], F32, tag="el")
                nc.scalar.activation(el[:, :, :], L3[:, :, :], AF.Exp)
                elm = gpool.tile([128, NT_, E], F32, tag="elm")
                nc.vector.tensor_tensor(elm[:, :, :], el[:, :, :], sel[:, :, :], op=OP.mult)
                zg = gpool.tile([128, NT_, 1], F32, tag="zg")
                nc.vector.tensor_reduce(
                    zg[:, :, :], elm[:, :, :], axis=AX.X, op=OP.add,
                    opt_input=False, opt_output=False,
                )
                rzg = gpool.tile([128, NT_, 1], F32, tag="rzg")
                nc.vector.reciprocal(rzg[:, :, :], zg[:, :, :])
                nc.vector.tensor_tensor(
                    gp[:, :].rearrange("p (t e) -> p t e", e=E), elm[:, :, :],
                    rzg[:, :, :].broadcast_to((128, NT_, E)),
                    op=OP.mult,
                )
            if debug and max_phase >= 4:
                nc.sync.dma_start(dbg_gp[:, :], gp[:, :])

            # ================= phase 5: experts =================
            if max_phase >= 5:
             with (
                tc.tile_pool(name="outp", bufs=4) as opool,
                tc.tile_pool(name="wexp", bufs=1) as wpool,
                tc.tile_pool(name="ht", bufs=2) as hpool,
                tc.tile_pool(name="ps5", bufs=4, space="PSUM") as ps5h,
                tc.tile_pool(name="ps5b", bufs=4, space="PSUM") as ps5e,
            ):
                for e in range(E // 2, E):
                    t = wpool.tile([128, DH], F16, tag=f"w1_{e}", name=f"w1_{e}")
                    nc.gpsimd.dma_start(t[:, :], w1_d[e, :, :])
                    w1_all.append(t)
                    w2t = []
                    for mc, (h0, hs) in enumerate(MCH):
                        t = wpool.tile([hs, D], F16, tag=f"w2_{e}_{mc}", name=f"w2_{e}_{mc}")
                        nc.gpsimd.dma_start(t[:, :], w2_d[e, h0 : h0 + hs, :])
                        w2t.append(t)
                    w2_all.append(w2t)
                    if b2_nonzero:
                        t = wpool.tile([1, D], F16, tag=f"b2row{e}", name=f"b2row{e}")
                        nc.gpsimd.dma_start(t[:, :], b2_d[e, :, :])
                        b2_all.append(t)
                ones = None
                if b2_nonzero:
                    ones = cpool.tile([1, NTOK], F16, tag="ones")
                    nc.gpsimd.memset(ones[:, :], 1.0)
                for e in range(E):
                    w1t = w1_all[e]
                    w2t = w2_all[e]
                    b2row = b2_all[e] if b2_nonzero else None
                    ht = []
                    for mc, (h0, hs) in enumerate(MCH):
                        t = hpool.tile([hs, NTOK], F16, tag=f"ht{mc}")
                        ht.append(t)
                        for n, (n0, ns) in enumerate(N5):
                            hp = ps5h.tile([128, 480], F32, tag="hp")
                            nc.tensor.matmul(
                                hp[:hs, :ns], w1t[:, h0 : h0 + hs],
                                attnT[:, n0 : n0 + ns],
                                start=True, stop=True,
                            )
                            nc.scalar.activation(
                                t[:, n0 : n0 + ns], hp[:hs, :ns], AF.Relu,
                                bias=b1_s[:hs, e * 5 + mc : e * 5 + mc + 1], scale=1.0,
                            )
                    for ti, (t0, ts_) in enumerate(TOK):
                        eo = ps5e.tile([128, D], F32, tag="eo")
                        for mc in range(5):
                            nc.tensor.matmul(
                                eo[:ts_, :], ht[mc][:, t0 : t0 + ts_], w2t[mc][:, :],
                                start=(mc == 0), stop=(mc == 4 and not b2_nonzero),
                            )
                        if b2_nonzero:
                            nc.tensor.matmul(
                                eo[:ts_, :], ones[:, t0 : t0 + ts_], b2row[:, :],
                                start=False, stop=True,
                            )
                        gsc = gp[:ts_, ti * E + e : ti * E + e + 1]
                        gsl = G[:ts_, ti * D : (ti + 1) * D]
                        ssl = S[:ts_, ti * D : (ti + 1) * D]
                        if e == 0:
                            nc.vector.tensor_scalar_mul(gsl, eo[:ts_, :], gsc)
                            nc.scalar.copy(ssl, eo[:ts_, :])
                        else:
                            nc.vector.scalar_tensor_tensor(
                                gsl, eo[:ts_, :], gsc, gsl, op0=OP.mult, op1=OP.add
                            )
                            nc.vector.tensor_tensor(ssl, eo[:ts_, :], ssl, op=OP.add)
                        if e == E - 1:
                            o = opool.tile([128, D], F32, tag="o")
                            nc.vector.tensor_tensor(o[:ts_, :], gsl, ssl, op=OP.mult)
                            nc.sync.dma_start(out_d[t0 : t0 + ts_, :], o[:ts_, :])

            _late.close()

    _split_waits(nc)
    return nc


_CACHE = {}
LAST_RESULT = None


def _get_module(b2_nonzero: bool, debug: bool = False, max_phase: int = 9):
    key = (b2_nonzero, debug, max_phase)
    if key not in _CACHE:
        _CACHE[key] = _build_module(b2_nonzero, debug=debug, max_phase=max_phase)
    return _CACHE[key]


def _prep_inputs(x, wh_bias, wh_W, Wq, Wk, Wv, proj_W, proj_b,
                 exp_W1, exp_b1, exp_W2, exp_b2, w_gate):
    # fold whiten into qkv; fold attention scale into q
    scale = float(D) ** -0.5
    Wqf = (Wq.reshape(DHK, D) @ wh_W) * scale          # [64, 768]
    Wkf = Wk.reshape(DHK, D) @ wh_W
    Wvf = Wv.reshape(DHK, D) @ wh_W

    def stripe(w):                                     # [64, 768] -> [128, 768]
        out = np.zeros((128, DIN), np.float32)
        for h in range(H):
            out[h * 32 : h * 32 + HS] = w[h * HS : (h + 1) * HS]
        return out

    def stripe_b(v):                                   # [64] -> [128]
        out = np.zeros(128, np.float32)
        for h in range(H):
            out[h * 32 : h * 32 + HS] = v[h * HS : (h + 1) * HS]
        return out

    wqkv = np.concatenate([stripe(Wqf), stripe(Wkf), Wvf], 0)   # [320, 768]
    bq = -(Wqf @ wh_bias)
    bk = -(Wkf @ wh_bias)
    bv = -(Wvf @ wh_bias)
    qkvb = np.stack([stripe_b(bq), stripe_b(bk), np.pad(bv, (0, 64))], 1)  # [128, 3]

    # head-striped proj weights [128, D]: rows 32h:32h+16 = head h, zeros
    # elsewhere (attnT garbage rows are the constant Z/Z = 1 there)
    projwS = np.zeros((128, D), np.float64)
    for h in range(H):
        projwS[32 * h : 32 * h + HS] = proj_W[:, h * HS : (h + 1) * HS].T

    # fold proj into the gate and expert W1 weights (fp64 host math)
    wgf = projwS @ w_gate.astype(np.float64)               # [128, E]
    cb = proj_b.astype(np.float64) @ w_gate.astype(np.float64)   # [E]
    w1f = np.einsum(
        "pd,ehd->eph", projwS, exp_W1.astype(np.float64)
    )                                                      # [E, 128, 600]
    b1f = exp_W1.astype(np.float64) @ proj_b.astype(np.float64) + exp_b1  # [E, 600]

    # b1 bias slices [128, E*5]: column e*5+mc holds b1f[e, h0:h0+hs]
    b1s = np.zeros((128, E * 5), np.float32)
    for e in range(E):
        for mc, (h0, hs) in enumerate(MCH):
            b1s[:hs, e * 5 + mc] = b1f[e, h0 : h0 + hs]

    f16 = np.float16
    common = {
        "wqkv": np.ascontiguousarray(wqkv.T).astype(f16),
        "qkvb": np.ascontiguousarray(qkvb).astype(np.float32),
        "wgf": np.ascontiguousarray(wgf).astype(f16),
        "cb": np.ascontiguousarray(cb[:, None]).astype(np.float32),
        "idm": np.eye(128, dtype=f16),
        "w1t": np.ascontiguousarray(w1f).astype(f16),
        "w2t": np.ascontiguousarray(exp_W2.transpose(0, 2, 1)).astype(f16),
        "b1s": b1s.astype(np.float32),
        "b2s": np.ascontiguousarray(exp_b2[:, None, :]).astype(f16),
    }
    in_maps = []
    for c in range(NCORES):
        xc = x[c * BL : (c + 1) * BL]                  # [8, 300, 768]
        xt = np.ascontiguousarray(
            xc.transpose(2, 0, 1).reshape(DIN, NTOK)
        ).astype(f16)
        in_maps.append({**common, "xt": xt})
    return in_maps


def kernel(x, wh_bias, wh_W, Wq, Wk, Wv, proj_W, proj_b,
           exp_W1, exp_b1, exp_W2, exp_b2, w_gate,
           debug=False, max_phase=9):
    global LAST_RESULT
    x = np.asarray(x, np.float32)
    wh_bias = np.asarray(wh_bias, np.float32)
    wh_W = np.asarray(wh_W, np.float32)
    Wq, Wk, Wv = (np.asarray(w, np.float32) for w in (Wq, Wk, Wv))
    proj_W = np.asarray(proj_W, np.float32)
    proj_b = np.asarray(proj_b, np.float32)
    exp_W1 = np.asarray(exp_W1, np.float32)
    exp_b1 = np.asarray(exp_b1, np.float32)
    exp_W2 = np.asarray(exp_W2, np.float32)
    exp_b2 = np.asarray(exp_b2, np.float32)
    w_gate = np.asarray(w_gate, np.float32)

    b2_nonzero = bool(np.any(exp_b2))
    in_maps = _prep_inputs(x, wh_bias, wh_W, Wq, Wk, Wv, proj_W, proj_b,
                           exp_W1, exp_b1, exp_W2, exp_b2, w_gate)

    nc = _get_module(b2_nonzero, debug=debug, max_phase=max_phase)
    for alloc in nc.m.functions[0].allocations:
        if isinstance(alloc, mybir.MemoryLocationSet) and alloc.kind == "ExternalInput":
            nm = alloc.memorylocations[0].name
            if nm not in in_maps[0]:
                continue  # partition_id etc., supplied by the runner
            got = in_maps[0][nm]
            assert tuple(got.shape) == tuple(alloc.tensor_shape), (
                nm, got.shape, alloc.tensor_shape)
            assert got.dtype == mybir.dt.np(alloc.dtype), (nm, got.dtype)
    res = run_bass_kernel_spmd(nc, in_maps, core_ids=list(range(NCORES)))
    LAST_RESULT = res
    out = np.stack([r["out"] for r in res.results])    # [8, 2400, 300]
    return out.reshape(B, T, D)


# revision 35
# speedup vs baseline: 1.0375x; 1.0265x over previous
"""Trainium2 Bass kernel for nn_MoEAdaptorLayer (whiten -> causal MHA -> proj
-> noisy-top-k gating (eval) -> 8 dense experts -> gated mixture * expert sum).

Sharding: data-parallel over batch. 64 batches -> 8 per core, params replicated.

v2 redesign vs baseline:
- fp16 everywhere on the matmul path (fp32 PSUM accumulation); halves DMA and
  removes the fp32r small-free-dim PE penalties.
- qkv: x shipped as one [768, 2400] fp16 tensor per core (6 big DMAs), matmuls
  in [128, 480] chunks.
- attention: single-orientation flash-style pass. Scores computed [s, t] only;
  exp on scalar; causal mask by multiplying the diagonal blocks with a
  triangular constant (split vector/gpsimd). AV and the softmax denominator
  come from ONE matmul per (head, s-chunk): stationary [v_h | ones] (32 cols)
  so the psum holds [av_h (16 rows) | Z_h replicated (16 rows)] per 32-stripe.
  One reciprocal + one fused min-mult STT per batch produce attnT directly in
  head-striped [128, tok] layout (no Z-pass, no output transposes).
- proj: stationary is zero-padded to the 32-stripe layout; 15 fp16 matmuls.
- gating: logits computed transposed ([8, tok], weight-stationary, 15 matmuls
  instead of 57 LDW-bound ones), PE-transposed back in 19 tiny transposes.
- experts: h chunked {128,128,128,128,88} (M=128-aligned LDWEIGHTS is ~2x
  faster than M=120), fp16 weights/activations.
"""

from contextlib import ExitStack

import numpy as np

import concourse.bass as bass
import concourse.tile as tile
import concourse.mybir as mybir
from concourse.bass_utils import run_bass_kernel_spmd

F16 = mybir.dt.float16
F32 = mybir.dt.float32
AX = mybir.AxisListType
OP = mybir.AluOpType
AF = mybir.ActivationFunctionType

B, T, DIN, D, E, H, HS = 64, 300, 768, 300, 8, 4, 16
NCORES = 8
BL = B // NCORES          # 8 batches per core
NTOK = BL * T             # 2400 tokens per core
DHK = H * HS              # 64
DH = 2 * D                # 600
TOPK = E // 2

K768 = [(i * 128, 128) for i in range(6)]
DC = [(0, 128), (128, 128), (256, 44)]          # 300 = 128+128+44
MCH = [(0, 128), (128, 128), (256, 128), (384, 128), (512, 88)]  # 600
TB = [(0, 128), (128, 128), (256, 44)]          # 300 tokens per batch
TOK = [(i * 128, 128) for i in range(18)] + [(2304, 96)]   # 2400 tokens
N5 = [(i * 480, 480) for i in range(5)]         # 2400 free-dim chunks
NEG = -1.0e30
RCLAMP = 16.0   # cap on 1/Z; keeps garbage stripe rows finite in fp16

_MAX_DRAIN_WAITS = 1
_WAIT_LIMIT = 1


def _split_waits(nc):
    """Walrus in this build caps sync waits per instruction; hoist excess
    waits onto same-engine NOPs inserted just before the instruction."""
    n = 0
    for f in nc.m.functions:
        for blk in f.blocks:
            insts = blk.instructions
            out = []
            changed = False
            for inst in insts:
                si = inst.sync_info
                waits = list(si.on_wait or []) if si is not None else []
                if len(waits) > _WAIT_LIMIT:
                    head, tail = waits[:-_WAIT_LIMIT], waits[-_WAIT_LIMIT:]
                    for i in range(0, len(head), _WAIT_LIMIT):
                        n += 1
                        nop = mybir.InstNoOp(name=f"waitnop{n}", ins=[], outs=[])
                        nop.engine = inst.engine
                        nop.sync_info = mybir.SyncInfo(
                            on_wait=head[i : i + _WAIT_LIMIT], on_update=[]
                        )
                        out.append(nop)
                    si.on_wait = tail
                    inst.sync_info = si
                    changed = True
                out.append(inst)
            if changed:
                blk.instructions = out


def _install_drain_patch():
    """This walrus build rejects CTRL instructions with more than a few sync
    waits; Tile's tail drain waits on every engine/DMA semaphore at once.
    Split the waits across a chain of single-wait drains."""
    if getattr(tile.TileContext, "_drain_patched", False):
        return

    def _patched(self, tick_clock, wait_clock):
        nc = self.nc
        drain_inst = nc.sync.drain()
        wait_clock.add_sem_waits(
            drain_inst.ins, tile.ScopedClock({None: tick_clock.global_clock})
        )
        ri = drain_inst.ins
        si = ri.sync_info
        waits = list(si.on_wait or []) if si is not None else []
        if len(waits) > _MAX_DRAIN_WAITS:
            si.on_wait = waits[:_MAX_DRAIN_WAITS]
            ri.sync_info = si
            for i in range(_MAX_DRAIN_WAITS, len(waits), _MAX_DRAIN_WAITS):
                d2 = nc.sync.drain()
                d2.ins.sync_info = mybir.SyncInfo(
                    on_wait=waits[i : i + _MAX_DRAIN_WAITS], on_update=[]
                )
        nc.all_engine_barrier()
        assert self.sems is not None
        popped = nc._tile_sem_poison_stack.pop()
        assert popped is self._sem_poison
        nc.clear_and_free_semaphores(list(self.sems.allocated().values()))
        nc.all_engine_barrier()

    tile.TileContext._drain_and_barrier = _patched
    tile.TileContext._drain_patched = True


def _build_module(b2_nonzero: bool, debug: bool = False, max_phase: int = 9):
    _install_drain_patch()
    nc = bass.Bass("TRN2", target_bir_lowering=False, debug=False)

    # ---- DRAM I/O ----
    xt_d = nc.dram_tensor("xt", [DIN, NTOK], F16, kind="ExternalInput")
    wqkv_d = nc.dram_tensor("wqkv", [DIN, 320], F16, kind="ExternalInput")
    qkvb_d = nc.dram_tensor("qkvb", [128, 3], F32, kind="ExternalInput")
    wgf_d = nc.dram_tensor("wgf", [128, E], F16, kind="ExternalInput")
    cb_d = nc.dram_tensor("cb", [E, 1], F32, kind="ExternalInput")
    w1_d = nc.dram_tensor("w1t", [E, 128, DH], F16, kind="ExternalInput")
    w2_d = nc.dram_tensor("w2t", [E, DH, D], F16, kind="ExternalInput")
    b1_d = nc.dram_tensor("b1s", [128, E * 5], F32, kind="ExternalInput")
    b2_d = nc.dram_tensor("b2s", [E, 1, D], F16, kind="ExternalInput")
    idm_d = nc.dram_tensor("idm", [128, 128], F16, kind="ExternalInput")
    out_d = nc.dram_tensor("out", [NTOK, D], F32, kind="ExternalOutput")
    if debug:
        dbg_q = nc.dram_tensor("dbg_q", [128, NTOK], F32, kind="ExternalOutput")
        dbg_k = nc.dram_tensor("dbg_k", [128, NTOK], F32, kind="ExternalOutput")
        dbg_at = nc.dram_tensor("dbg_at", [128, NTOK], F32, kind="ExternalOutput")
        dbg_xa = nc.dram_tensor("dbg_xa", [D, NTOK], F32, kind="ExternalOutput")
        dbg_gp = nc.dram_tensor("dbg_gp", [128, len(TOK) * E], F32, kind="ExternalOutput")

    with tile.TileContext(nc) as tc:
        with (
            tc.tile_pool(name="const", bufs=1) as cpool,
            tc.tile_pool(name="big", bufs=1) as bpool,
        ):
            # ---- persistent constants ----
            ident = cpool.tile([128, 128], F16)
            wqkv_s = []
            for kc, (k0, ks) in enumerate(K768):
                t = cpool.tile([128, 320], F16, tag=f"wqkv{kc}")
                nc.sync.dma_start(t[:, :], wqkv_d[k0 : k0 + ks, :])
                wqkv_s.append(t)
            qkvb_s = cpool.tile([128, 3], F32, tag="qkvbs")
            nc.sync.dma_start(qkvb_s[:, :], qkvb_d[:, :])
            wgf_s = cpool.tile([128, E], F16, tag="wgf")
            nc.sync.dma_start(wgf_s[:, :], wgf_d[:, :])
            cb_s = cpool.tile([E, 1], F32, tag="cb")
            nc.sync.dma_start(cb_s[:, :], cb_d[:, :])
            b1_s = cpool.tile([128, E * 5], F32, tag="b1s")
            nc.sync.dma_start(b1_s[:, :], b1_d[:, :])
            nc.sync.dma_start(ident[:, :], idm_d[:, :])

            # x input on the gpsimd queue ahead of the expert weights
            xt_s = [
                bpool.tile([128, NTOK], F16, tag=f"xt{kc}", name=f"xt{kc}")
                for kc in range(6)
            ]
            for kc, (k0, ks) in enumerate(K768):
                nc.gpsimd.dma_start(xt_s[kc][:, :], xt_d[k0 : k0 + ks, :])

            # prefetch experts 0-3 weights up front on the gpsimd DMA queue
            w1_all, w2_all, b2_all = [], [], []
            for e in range(E // 2):
                t = bpool.tile([128, DH], F16, tag=f"w1_{e}", name=f"w1_{e}")
                nc.gpsimd.dma_start(t[:, :], w1_d[e, :, :])
                w1_all.append(t)
                w2t = []
                for mc, (h0, hs) in enumerate(MCH):
                    t = bpool.tile([hs, D], F16, tag=f"w2_{e}_{mc}", name=f"w2_{e}_{mc}")
                    nc.gpsimd.dma_start(t[:, :], w2_d[e, h0 : h0 + hs, :])
                    w2t.append(t)
                w2_all.append(w2t)
                if b2_nonzero:
                    t = bpool.tile([1, D], F16, tag=f"b2row{e}", name=f"b2row{e}")
                    nc.gpsimd.dma_start(t[:, :], b2_d[e, :, :])
                    b2_all.append(t)

            # ---- activations ----
            _mid = ExitStack()
            mpool = _mid.enter_context(tc.tile_pool(name="mid", bufs=1))
            # heads striped at 32-partition bases (rows 32h:32h+16 per head)
            qTs = mpool.tile([128, NTOK], F16, tag="qTs")
            kTs = mpool.tile([128, NTOK], F16, tag="kTs")
            qTb = mpool.tile([DHK, NTOK], F16, tag="qTb")
            kTb = mpool.tile([DHK, NTOK], F16, tag="kTb")
            vT = mpool.tile([DHK, NTOK], F16, tag="vT")
            # v_s[sc]: [ss, b, h, 64] = [v_h(16) | 0(16) | ones(16) | 0(16)];
            # the av/Z layout this produces keeps every vector-op partition
            # base 32-aligned
            v_s = [
                mpool.tile([128, BL, H, 64], F16, tag=f"v{sc}", name=f"v{sc}")
                for sc in range(3)
            ]
            # attnT [128, NTOK]: rows 32h:32h+16 = head h, other rows are
            # the constant 1.0 (Z/Z); folded weights are zero there
            attnT = bpool.tile([128, NTOK], F16, tag="attnT")
            # v_s constant columns [v16 | ones48]: one matmul per (h, sc)
            # produces [av (16 rows) | Z replicated (48 rows)]
            for sc in range(3):
                nc.gpsimd.memset(v_s[sc][:, :, :, 16:64], 1.0)

            # ================= phase 1: qkv =================
            with (
                tc.tile_pool(name="ps1", bufs=4, space="PSUM") as ps1,
            ):

                specs = ((qTs, 128, 0), (kTs, 128, 128), (vT, DHK, 256))
                for n0, ns in N5:
                    for mi, (dst, mw, c0) in enumerate(specs):
                        ps = ps1.tile([128, 480], F32, tag="qkvp")
                        for kc in range(6):
                            nc.tensor.matmul(
                                ps[:mw, :ns],
                                wqkv_s[kc][:, c0 : c0 + mw],
                                xt_s[kc][:, n0 : n0 + ns],
                                start=(kc == 0), stop=(kc == 5),
                            )
                        nc.scalar.activation(
                            dst[:, n0 : n0 + ns], ps[:mw, :ns], AF.Identity,
                            bias=qkvb_s[:mw, mi : mi + 1], scale=1.0,
                        )
                # move heads 2-3 down to base 0/32 (DMA shifts partitions)
                nc.sync.dma_start(qTb[:, :], qTs[64:128, :])
                nc.sync.dma_start(kTb[:, :], kTs[64:128, :])
                if debug:
                    dq = mpool.tile([128, NTOK], F32, tag="dbgq")
                    nc.vector.tensor_copy(dq[:, :], qTs[:, :])
                    nc.sync.dma_start(dbg_q[:, :], dq[:, :])
                    dk = mpool.tile([128, NTOK], F32, tag="dbgk")
                    nc.vector.tensor_copy(dk[:, :], kTs[:, :])
                    nc.sync.dma_start(dbg_k[:, :], dk[:, :])

            # ================= phase 2: attention =================
            if max_phase >= 2:
             with (
                tc.tile_pool(name="att", bufs=2) as apool,
                tc.tile_pool(name="ps2", bufs=2, space="PSUM") as ps2,
                tc.tile_pool(name="ps2z", bufs=2, space="PSUM") as ps2z,
            ):
                for b in range(BL):
                    base = b * T
                    # v = vT.T per s-chunk (all heads at once)
                    for sc, (s0, ss) in enumerate(TB):
                        vtr = ps2.tile([128, DHK], F16, tag="vtr")
                        nc.tensor.transpose(
                            vtr[:ss, :], vT[:, base + s0 : base + s0 + ss],
                            ident[0:DHK, 0:DHK],
                        )
                        nc.scalar.copy(
                            v_s[sc][0:ss, b, :, 0:16],
                            vtr[:ss, :].rearrange("p (h k) -> p h k", h=H),
                        )
                    # FT pass: scores [s, t] with the causal -30000 mask added
                    # into the diag block via an identity-stationary matmul;
                    # exp; then one matmul per (h, sc) computing
                    # [av_h | 0 | Z_h | 0] in a 64-row block; heads 2g / 2g+1
                    # at row bases 0 / 64 of azb[g]
                    azb = [
                        ps2z.tile([128, T], F32, tag=f"azb{g}", name=f"azb{g}")
                        for g in range(2)
                    ]
                    ft = {}
                    for h in range(H):
                        qt = qTs if h < 2 else qTb
                        kt = kTs if h < 2 else kTb
                        q0 = k0 = (h % 2) * 32
                        for sc, (s0, ss) in enumerate(TB):
                            tlen = T - s0
                            weiT = ps2.tile([128, T], F32, tag="weiT")
                            nc.tensor.matmul(
                                weiT[:ss, :tlen],
                                kt[k0 : k0 + HS, base + s0 : base + s0 + ss],
                                qt[q0 : q0 + HS, base + s0 : base + T],
                                start=True, stop=True,
                            )
                            f = apool.tile([128, T], F16, tag=f"ft{h}{sc}")
                            ft[(h, sc)] = f
                            nc.scalar.activation(f[:ss, :tlen], weiT[:ss, :tlen], AF.Exp)
                            nc.gpsimd.affine_select(
                                out=f[:ss, :ss], in_=f[:ss, :ss],
                                compare_op=OP.is_ge, fill=0.0,
                                base=0, channel_multiplier=-1, pattern=[[1, ss]],
                            )
                        for sc, (s0, ss) in enumerate(TB):
                            nc.tensor.matmul(
                                azb[h // 2][64 * (h % 2) : 64 * (h % 2) + 64, s0:T],
                                v_s[sc][0:ss, b, h, :],
                                ft[(h, sc)][0:ss, 0 : T - s0],
                                start=(sc == 0), stop=(sc == 2),
                            )
                    # 1/azb on the scalar table engine (cheap); only the
                    # Z-recip rows are ever read back, so the garbage
                    # av-recip rows (possibly inf) are harmless
                    rz = [
                        apool.tile([128, T], F32, tag=f"rz{g}", name=f"rz{g}")
                        for g in range(2)
                    ]
                    for g in range(2):
                        nc.vector.reciprocal(rz[g][:, :], azb[g][:, :])
                    for h in range(H):
                        g, o = h // 2, 64 * (h % 2)
                        nc.vector.scalar_tensor_tensor(
                            attnT[32 * h : 32 * h + 32, base : base + T],
                            rz[g][o + 32 : o + 64, :], RCLAMP,
                            azb[g][o : o + 32, :],
                            op0=OP.min, op1=OP.mult,
                        )
                if debug:
                    da = apool.tile([128, NTOK], F32, tag="dbga")
                    nc.vector.tensor_copy(da[:, :], attnT[:, :])
                    nc.sync.dma_start(dbg_at[:, :], da[:, :])

            _mid.close()
            _late = ExitStack()
            lpool = _late.enter_context(tc.tile_pool(name="late", bufs=1))
            gp = lpool.tile([128, len(TOK) * E], F32, tag="gp")
            G = lpool.tile([128, len(TOK) * D], F32, tag="G")
            S = lpool.tile([128, len(TOK) * D], F32, tag="S")

            # ================= phase 4: gating =================
            if max_phase >= 4:
             with (
                tc.tile_pool(name="gat", bufs=1) as gpool,
                tc.tile_pool(name="ps4", bufs=2, space="PSUM") as ps4,
                tc.tile_pool(name="ps4t", bufs=1, space="PSUM") as ps4t,
             ):
                NT_ = len(TOK)
                # logits transposed: [8, tok], folded proj+gate stationary
                logT = gpool.tile([8, NTOK], F16, tag="logT")
                for n0, ns in N5:
                    lg = ps4.tile([8, 480], F32, tag="lg")
                    nc.tensor.matmul(
                        lg[:, :ns], wgf_s[:, :], attnT[:, n0 : n0 + ns],
                        start=True, stop=True,
                    )
                    nc.scalar.activation(
                        logT[:, n0 : n0 + ns], lg[:, :ns], AF.Identity,
                        bias=cb_s[:, :], scale=1.0,
                    )
                # transpose back to [tok, 8] in one psum bank
                ltp = ps4t.tile([128, NT_, E], F16, tag="ltp")
                for ti, (t0, ts_) in enumerate(TOK):
                    nc.tensor.transpose(
                        ltp[:ts_, ti, :], logT[:, t0 : t0 + ts_], ident[0:8, 0:8]
                    )
                L3 = gpool.tile([128, NT_, E], F32, tag="L3")
                nc.scalar.copy(L3[:, :, :], ltp[:, :, :])
                work = gpool.tile([128, NT_, E], F32, tag="work")
                nc.vector.tensor_copy(work[:, :, :], L3[:, :, :])
                m = gpool.tile([128, NT_, 4], F32, tag="m")
                eqm = gpool.tile([128, NT_, E], F32, tag="eqm")
                for r in range(TOPK):
                    nc.vector.tensor_reduce(
                        m[:, :, r : r + 1], work[:, :, :], axis=AX.X, op=OP.max,
                        opt_input=False, opt_output=False,
                    )
                    if r < TOPK - 1:
                        nc.vector.tensor_tensor(
                            eqm[:, :, :], work[:, :, :],
                            m[:, :, r : r + 1].broadcast_to((128, NT_, E)),
                            op=OP.is_equal,
                        )
                        nc.vector.scalar_tensor_tensor(
                            work[:, :, :], eqm[:, :, :], NEG, work[:, :, :],
                            op0=OP.mult, op1=OP.add,
                        )
                sel = gpool.tile([128, NT_, E], F32, tag="sel")
                nc.vector.tensor_tensor(
                    sel[:, :, :], L3[:, :, :],
                    m[:, :, 3:4].broadcast_to((128, NT_, E)), op=OP.is_ge,
                )
                el = gpool.tile([128, NT_, E], F32, tag="el")
                nc.scalar.activation(el[:, :, :], L3[:, :, :], AF.Exp)
                elm = gpool.tile([128, NT_, E], F32, tag="elm")
                nc.vector.tensor_tensor(elm[:, :, :], el[:, :, :], sel[:, :, :], op=OP.mult)
                zg = gpool.tile([128, NT_, 1], F32, tag="zg")
                nc.vector.tensor_reduce(
                    zg[:, :, :], elm[:, :, :], axis=AX.X, op=OP.add,
                    opt_input=False, opt_output=False,
                )
                rzg = gpool.tile([128, NT_, 1], F32, tag="rzg")
                nc.vector.reciprocal(rzg[:, :, :], zg[:, :, :])
                nc.vector.tensor_tensor(
                    gp[:, :].rearrange("p (t e) -> p t e", e=E), elm[:, :, :],
                    rzg[:, :, :].broadcast_to((128, NT_, E)),
                    op=OP.mult,
                )
            if debug and max_phase >= 4:
                nc.sync.dma_start(dbg_gp[:, :], gp[:, :])

            # ================= phase 5: experts =================
            if max_phase >= 5:
             with (
                tc.tile_pool(name="outp", bufs=4) as opool,
                tc.tile_pool(name="wexp", bufs=1) as wpool,
                tc.tile_pool(name="ht", bufs=2) as hpool,
                tc.tile_pool(name="ps5", bufs=4, space="PSUM") as ps5h,
                tc.tile_pool(name="ps5b", bufs=4, space="PSUM") as ps5e,
            ):
                for e in range(E // 2, E):
                    t = wpool.tile([128, DH], F16, tag=f"w1_{e}", name=f"w1_{e}")
                    nc.gpsimd.dma_start(t[:, :], w1_d[e, :, :])
                    w1_all.append(t)
                    w2t = []
                    for mc, (h0, hs) in enumerate(MCH):
                        t = wpool.tile([hs, D], F16, tag=f"w2_{e}_{mc}", name=f"w2_{e}_{mc}")
                        nc.gpsimd.dma_start(t[:, :], w2_d[e, h0 : h0 + hs, :])
                        w2t.append(t)
                    w2_all.append(w2t)
                    if b2_nonzero:
                        t = wpool.tile([1, D], F16, tag=f"b2row{e}", name=f"b2row{e}")
                        nc.gpsimd.dma_start(t[:, :], b2_d[e, :, :])
                        b2_all.append(t)
                ones = None
                if b2_nonzero:
                    ones = cpool.tile([1, NTOK], F16, tag="ones")
                    nc.gpsimd.memset(ones[:, :], 1.0)
                for e in range(E):
                    w1t = w1_all[e]
                    w2t = w2_all[e]
                    b2row = b2_all[e] if b2_nonzero else None
                    ht = []
                    for mc, (h0, hs) in enumerate(MCH):
                        t = hpool.tile([hs, NTOK], F16, tag=f"ht{mc}")
                        ht.append(t)
                        for n, (n0, ns) in enumerate(N5):
                            hp = ps5h.tile([128, 480], F32, tag="hp")
                            nc.tensor.matmul(
                                hp[:hs, :ns], w1t[:, h0 : h0 + hs],
                                attnT[:, n0 : n0 + ns],
                                start=True, stop=True,
                            )
                            nc.scalar.activation(
                                t[:, n0 : n0 + ns], hp[:hs, :ns], AF.Relu,
                                bias=b1_s[:hs, e * 5 + mc : e * 5 + mc + 1], scale=1.0,
                            )
                    for ti, (t0, ts_) in enumerate(TOK):
                        eo = ps5e.tile([128, D], F32, tag="eo")
                        for mc in range(5):
                            nc.tensor.matmul(
                                eo[:ts_, :], ht[mc][:, t0 : t0 + ts_], w2t[mc][:, :],
                                start=(mc == 0), stop=(mc == 4 and not b2_nonzero),
                            )
                        if b2_nonzero:
                            nc.tensor.matmul(
                                eo[:ts_, :], ones[:, t0 : t0 + ts_], b2row[:, :],
                                start=False, stop=True,
                            )
                        gsc = gp[:ts_, ti * E + e : ti * E + e + 1]
                        gsl = G[:ts_, ti * D : (ti + 1) * D]
                        ssl = S[:ts_, ti * D : (ti + 1) * D]
                        if e == 0:
                            nc.vector.tensor_scalar_mul(gsl, eo[:ts_, :], gsc)
                            nc.scalar.copy(ssl, eo[:ts_, :])
                        else:
                            nc.vector.scalar_tensor_tensor(
                                gsl, eo[:ts_, :], gsc, gsl, op0=OP.mult, op1=OP.add
                            )
                            nc.vector.tensor_tensor(ssl, eo[:ts_, :], ssl, op=OP.add)
                        if e == E - 1:
                            o = opool.tile([128, D], F32, tag="o")
                            nc.vector.tensor_tensor(o[:ts_, :], gsl, ssl, op=OP.mult)
                            nc.sync.dma_start(out_d[t0 : t0 + ts_, :], o[:ts_, :])

            _late.close()

    _split_waits(nc)
    return nc


_CACHE = {}
LAST_RESULT = None


def _get_module(b2_nonzero: bool, debug: bool = False, max_phase: int = 9):
    key = (b2_nonzero, debug, max_phase)
    if key not in _CACHE:
        _CACHE[key] = _build_module(b2_nonzero, debug=debug, max_phase=max_phase)
    return _CACHE[key]


def _prep_inputs(x, wh_bias, wh_W, Wq, Wk, Wv, proj_W, proj_b,
                 exp_W1, exp_b1, exp_W2, exp_b2, w_gate):
    # fold whiten into qkv; fold attention scale into q
    scale = float(D) ** -0.5
    Wqf = (Wq.reshape(DHK, D) @ wh_W) * scale          # [64, 768]
    Wkf = Wk.reshape(DHK, D) @ wh_W
    Wvf = Wv.reshape(DHK, D) @ wh_W

    def stripe(w):                                     # [64, 768] -> [128, 768]
        out = np.zeros((128, DIN), np.float32)
        for h in range(H):
            out[h * 32 : h * 32 + HS] = w[h * HS : (h + 1) * HS]
        return out

    def stripe_b(v):                                   # [64] -> [128]
        out = np.zeros(128, np.float32)
        for h in range(H):
            out[h * 32 : h * 32 + HS] = v[h * HS : (h + 1) * HS]
        return out

    wqkv = np.concatenate([stripe(Wqf), stripe(Wkf), Wvf], 0)   # [320, 768]
    bq = -(Wqf @ wh_bias)
    bk = -(Wkf @ wh_bias)
    bv = -(Wvf @ wh_bias)
    qkvb = np.stack([stripe_b(bq), stripe_b(bk), np.pad(bv, (0, 64))], 1)  # [128, 3]

    # head-striped proj weights [128, D]: rows 32h:32h+16 = head h, zeros
    # elsewhere (attnT garbage rows are the constant Z/Z = 1 there)
    projwS = np.zeros((128, D), np.float64)
    for h in range(H):
        projwS[32 * h : 32 * h + HS] = proj_W[:, h * HS : (h + 1) * HS].T

    # fold proj into the gate and expert W1 weights (fp64 host math)
    wgf = projwS @ w_gate.astype(np.float64)               # [128, E]
    cb = proj_b.astype(np.float64) @ w_gate.astype(np.float64)   # [E]
    w1f = np.einsum(
        "pd,ehd->eph", projwS, exp_W1.astype(np.float64)
    )                                                      # [E, 128, 600]
    b1f = exp_W1.astype(np.float64) @ proj_b.astype(np.float64) + exp_b1  # [E, 600]

    # b1 bias slices [128, E*5]: column e*5+mc holds b1f[e, h0:h0+hs]
    b1s = np.zeros((128, E * 5), np.float32)
    for e in range(E):
        for mc, (h0, hs) in enumerate(MCH):
            b1s[:hs, e * 5 + mc] = b1f[e, h0 : h0 + hs]

    f16 = np.float16
    common = {
        "wqkv": np.ascontiguousarray(wqkv.T).astype(f16),
        "qkvb": np.ascontiguousarray(qkvb).astype(np.float32),
        "wgf": np.ascontiguousarray(wgf).astype(f16),
        "cb": np.ascontiguousarray(cb[:, None]).astype(np.float32),
        "idm": np.eye(128, dtype=f16),
        "w1t": np.ascontiguousarray(w1f).astype(f16),
        "w2t": np.ascontiguousarray(exp_W2.transpose(0, 2, 1)).astype(f16),
        "b1s": b1s.astype(np.float32),
        "b2s": np.ascontiguousarray(exp_b2[:, None, :]).astype(f16),
    }
    in_maps = []
    for c in range(NCORES):
        xc = x[c * BL : (c + 1) * BL]                  # [8, 300, 768]
        xt = np.ascontiguousarray(
            xc.transpose(2, 0, 1).reshape(DIN, NTOK)
        ).astype(f16)
        in_maps.append({**common, "xt": xt})
    return in_maps


def kernel(x, wh_bias, wh_W, Wq, Wk, Wv, proj_W, proj_b,
           exp_W1, exp_b1, exp_W2, exp_b2, w_gate,
           debug=False, max_phase=9):
    global LAST_RESULT
    x = np.asarray(x, np.float32)
    wh_bias = np.asarray(wh_bias, np.float32)
    wh_W = np.asarray(wh_W, np.float32)
    Wq, Wk, Wv = (np.asarray(w, np.float32) for w in (Wq, Wk, Wv))
    proj_W = np.asarray(proj_W, np.float32)
    proj_b = np.asarray(proj_b, np.float32)
    exp_W1 = np.asarray(exp_W1, np.float32)
    exp_b1 = np.asarray(exp_b1, np.float32)
    exp_W2 = np.asarray(exp_W2, np.float32)
    exp_b2 = np.asarray(exp_b2, np.float32)
    w_gate = np.asarray(w_gate, np.float32)

    b2_nonzero = bool(np.any(exp_b2))
    in_maps = _prep_inputs(x, wh_bias, wh_W, Wq, Wk, Wv, proj_W, proj_b,
                           exp_W1, exp_b1, exp_W2, exp_b2, w_gate)

    nc = _get_module(b2_nonzero, debug=debug, max_phase=max_phase)
    for alloc in nc.m.functions[0].allocations:
        if isinstance(alloc, mybir.MemoryLocationSet) and alloc.kind == "ExternalInput":
            nm = alloc.memorylocations[0].name
            if nm not in in_maps[0]:
                continue  # partition_id etc., supplied by the runner
            got = in_maps[0][nm]
            assert tuple(got.shape) == tuple(alloc.tensor_shape), (
                nm, got.shape, alloc.tensor_shape)
            assert got.dtype == mybir.dt.np(alloc.dtype), (nm, got.dtype)
    res = run_bass_kernel_spmd(nc, in_maps, core_ids=list(range(NCORES)))
    LAST_RESULT = res
    out = np.stack([r["out"] for r in res.results])    # [8, 2400, 300]
    return out.reshape(B, T, D)


# revision 36
# speedup vs baseline: 1.0720x; 1.0333x over previous
"""Trainium2 Bass kernel for nn_MoEAdaptorLayer (whiten -> causal MHA -> proj
-> noisy-top-k gating (eval) -> 8 dense experts -> gated mixture * expert sum).

Sharding: data-parallel over batch. 64 batches -> 8 per core, params replicated.

v2 redesign vs baseline:
- fp16 everywhere on the matmul path (fp32 PSUM accumulation); halves DMA and
  removes the fp32r small-free-dim PE penalties.
- qkv: x shipped as one [768, 2400] fp16 tensor per core (6 big DMAs), matmuls
  in [128, 480] chunks.
- attention: single-orientation flash-style pass. Scores computed [s, t] only;
  exp on scalar; causal mask by multiplying the diagonal blocks with a
  triangular constant (split vector/gpsimd). AV and the softmax denominator
  come from ONE matmul per (head, s-chunk): stationary [v_h | ones] (32 cols)
  so the psum holds [av_h (16 rows) | Z_h replicated (16 rows)] per 32-stripe.
  One reciprocal + one fused min-mult STT per batch produce attnT directly in
  head-striped [128, tok] layout (no Z-pass, no output transposes).
- proj: stationary is zero-padded to the 32-stripe layout; 15 fp16 matmuls.
- gating: logits computed transposed ([8, tok], weight-stationary, 15 matmuls
  instead of 57 LDW-bound ones), PE-transposed back in 19 tiny transposes.
- experts: h chunked {128,128,128,128,88} (M=128-aligned LDWEIGHTS is ~2x
  faster than M=120), fp16 weights/activations.
"""

from contextlib import ExitStack

import numpy as np

import concourse.bass as bass
import concourse.tile as tile
import concourse.mybir as mybir
from concourse.bass_utils import run_bass_kernel_spmd

F16 = mybir.dt.float16
F32 = mybir.dt.float32
AX = mybir.AxisListType
OP = mybir.AluOpType
AF = mybir.ActivationFunctionType

B, T, DIN, D, E, H, HS = 64, 300, 768, 300, 8, 4, 16
NCORES = 8
BL = B // NCORES          # 8 batches per core
NTOK = BL * T             # 2400 tokens per core
DHK = H * HS              # 64
DH = 2 * D                # 600
TOPK = E // 2

K768 = [(i * 128, 128) for i in range(6)]
DC = [(0, 128), (128, 128), (256, 44)]          # 300 = 128+128+44
MCH = [(0, 128), (128, 128), (256, 128), (384, 128), (512, 88)]  # 600
TB = [(0, 128), (128, 128), (256, 44)]          # 300 tokens per batch
TOK = [(i * 128, 128) for i in range(18)] + [(2304, 96)]   # 2400 tokens
N5 = [(i * 480, 480) for i in range(5)]         # 2400 free-dim chunks
NEG = -1.0e30
RCLAMP = 16.0   # cap on 1/Z; keeps garbage stripe rows finite in fp16

_MAX_DRAIN_WAITS = 1
_WAIT_LIMIT = 1


def _split_waits(nc):
    """Walrus in this build caps sync waits per instruction; hoist excess
    waits onto same-engine NOPs inserted just before the instruction."""
    n = 0
    for f in nc.m.functions:
        for blk in f.blocks:
            insts = blk.instructions
            out = []
            changed = False
            for inst in insts:
                si = inst.sync_info
                waits = list(si.on_wait or []) if si is not None else []
                if len(waits) > _WAIT_LIMIT:
                    head, tail = waits[:-_WAIT_LIMIT], waits[-_WAIT_LIMIT:]
                    for i in range(0, len(head), _WAIT_LIMIT):
                        n += 1
                        nop = mybir.InstNoOp(name=f"waitnop{n}", ins=[], outs=[])
                        nop.engine = inst.engine
                        nop.sync_info = mybir.SyncInfo(
                            on_wait=head[i : i + _WAIT_LIMIT], on_update=[]
                        )
                        out.append(nop)
                    si.on_wait = tail
                    inst.sync_info = si
                    changed = True
                out.append(inst)
            if changed:
                blk.instructions = out


def _install_drain_patch():
    """This walrus build rejects CTRL instructions with more than a few sync
    waits; Tile's tail drain waits on every engine/DMA semaphore at once.
    Split the waits across a chain of single-wait drains."""
    if getattr(tile.TileContext, "_drain_patched", False):
        return

    def _patched(self, tick_clock, wait_clock):
        nc = self.nc
        drain_inst = nc.sync.drain()
        wait_clock.add_sem_waits(
            drain_inst.ins, tile.ScopedClock({None: tick_clock.global_clock})
        )
        ri = drain_inst.ins
        si = ri.sync_info
        waits = list(si.on_wait or []) if si is not None else []
        if len(waits) > _MAX_DRAIN_WAITS:
            si.on_wait = waits[:_MAX_DRAIN_WAITS]
            ri.sync_info = si
            for i in range(_MAX_DRAIN_WAITS, len(waits), _MAX_DRAIN_WAITS):
                d2 = nc.sync.drain()
                d2.ins.sync_info = mybir.SyncInfo(
                    on_wait=waits[i : i + _MAX_DRAIN_WAITS], on_update=[]
                )
        nc.all_engine_barrier()
        assert self.sems is not None
        popped = nc._tile_sem_poison_stack.pop()
        assert popped is self._sem_poison
        nc.clear_and_free_semaphores(list(self.sems.allocated().values()))
        nc.all_engine_barrier()

    tile.TileContext._drain_and_barrier = _patched
    tile.TileContext._drain_patched = True


def _build_module(b2_nonzero: bool, debug: bool = False, max_phase: int = 9):
    _install_drain_patch()
    nc = bass.Bass("TRN2", target_bir_lowering=False, debug=False)

    # ---- DRAM I/O ----
    xt_d = nc.dram_tensor("xt", [DIN, NTOK], F16, kind="ExternalInput")
    wqkv_d = nc.dram_tensor("wqkv", [DIN, 320], F16, kind="ExternalInput")
    qkvb_d = nc.dram_tensor("qkvb", [128, 3], F32, kind="ExternalInput")
    wgf_d = nc.dram_tensor("wgf", [128, E], F16, kind="ExternalInput")
    cb_d = nc.dram_tensor("cb", [E, 1], F32, kind="ExternalInput")
    w1_d = nc.dram_tensor("w1t", [E, 128, DH], F16, kind="ExternalInput")
    w2_d = nc.dram_tensor("w2t", [E, DH, D], F16, kind="ExternalInput")
    b1_d = nc.dram_tensor("b1s", [128, E * 5], F32, kind="ExternalInput")
    b2_d = nc.dram_tensor("b2s", [E, 1, D], F16, kind="ExternalInput")
    idm_d = nc.dram_tensor("idm", [128, 128], F16, kind="ExternalInput")
    out_d = nc.dram_tensor("out", [NTOK, D], F32, kind="ExternalOutput")
    if debug:
        dbg_q = nc.dram_tensor("dbg_q", [128, NTOK], F32, kind="ExternalOutput")
        dbg_k = nc.dram_tensor("dbg_k", [128, NTOK], F32, kind="ExternalOutput")
        dbg_at = nc.dram_tensor("dbg_at", [128, NTOK], F32, kind="ExternalOutput")
        dbg_xa = nc.dram_tensor("dbg_xa", [D, NTOK], F32, kind="ExternalOutput")
        dbg_gp = nc.dram_tensor("dbg_gp", [128, len(TOK) * E], F32, kind="ExternalOutput")

    with tile.TileContext(nc) as tc:
        with (
            tc.tile_pool(name="const", bufs=1) as cpool,
            tc.tile_pool(name="big", bufs=1) as bpool,
        ):
            # ---- persistent constants ----
            ident = cpool.tile([128, 128], F16)
            wqkv_s = []
            for kc, (k0, ks) in enumerate(K768):
                t = cpool.tile([128, 320], F16, tag=f"wqkv{kc}")
                nc.sync.dma_start(t[:, :], wqkv_d[k0 : k0 + ks, :])
                wqkv_s.append(t)
            qkvb_s = cpool.tile([128, 3], F32, tag="qkvbs")
            nc.sync.dma_start(qkvb_s[:, :], qkvb_d[:, :])
            wgf_s = cpool.tile([128, E], F16, tag="wgf")
            nc.sync.dma_start(wgf_s[:, :], wgf_d[:, :])
            cb_s = cpool.tile([E, 1], F32, tag="cb")
            nc.sync.dma_start(cb_s[:, :], cb_d[:, :])
            b1_s = cpool.tile([128, E * 5], F32, tag="b1s")
            nc.sync.dma_start(b1_s[:, :], b1_d[:, :])
            nc.sync.dma_start(ident[:, :], idm_d[:, :])

            # x input on the gpsimd queue ahead of the expert weights
            xt_s = [
                bpool.tile([128, NTOK], F16, tag=f"xt{kc}", name=f"xt{kc}")
                for kc in range(6)
            ]
            for kc, (k0, ks) in enumerate(K768):
                nc.gpsimd.dma_start(xt_s[kc][:, :], xt_d[k0 : k0 + ks, :])

            # prefetch experts 0-3 weights up front on the gpsimd DMA queue
            w1_all, w2_all, b2_all = [], [], []
            for e in range(E // 2):
                t = bpool.tile([128, DH], F16, tag=f"w1_{e}", name=f"w1_{e}")
                nc.gpsimd.dma_start(t[:, :], w1_d[e, :, :])
                w1_all.append(t)
                w2t = []
                for mc, (h0, hs) in enumerate(MCH):
                    t = bpool.tile([hs, D], F16, tag=f"w2_{e}_{mc}", name=f"w2_{e}_{mc}")
                    nc.gpsimd.dma_start(t[:, :], w2_d[e, h0 : h0 + hs, :])
                    w2t.append(t)
                w2_all.append(w2t)
                if b2_nonzero:
                    t = bpool.tile([1, D], F16, tag=f"b2row{e}", name=f"b2row{e}")
                    nc.gpsimd.dma_start(t[:, :], b2_d[e, :, :])
                    b2_all.append(t)

            # ---- activations ----
            _mid = ExitStack()
            mpool = _mid.enter_context(tc.tile_pool(name="mid", bufs=1))
            # heads striped at 32-partition bases (rows 32h:32h+16 per head)
            qTs = mpool.tile([128, NTOK], F16, tag="qTs")
            kTs = mpool.tile([128, NTOK], F16, tag="kTs")
            qTb = mpool.tile([DHK, NTOK], F16, tag="qTb")
            kTb = mpool.tile([DHK, NTOK], F16, tag="kTb")
            vT = mpool.tile([DHK, NTOK], F16, tag="vT")
            # v_s[sc]: [ss, b, h, 64] = [v_h(16) | 0(16) | ones(16) | 0(16)];
            # the av/Z layout this produces keeps every vector-op partition
            # base 32-aligned
            v_s = [
                mpool.tile([128, BL, H, 64], F16, tag=f"v{sc}", name=f"v{sc}")
                for sc in range(3)
            ]
            # attnT [128, NTOK]: rows 32h:32h+16 = head h, other rows are
            # the constant 1.0 (Z/Z); folded weights are zero there
            attnT = bpool.tile([128, NTOK], F16, tag="attnT")
            # v_s constant columns [v16 | ones48]: one matmul per (h, sc)
            # produces [av (16 rows) | Z replicated (48 rows)]
            for sc in range(3):
                nc.gpsimd.memset(v_s[sc][:, :, :, 16:64], 1.0)

            # ================= phase 1: qkv =================
            with (
                tc.tile_pool(name="ps1", bufs=4, space="PSUM") as ps1,
            ):

                specs = ((qTs, 128, 0), (kTs, 128, 128), (vT, DHK, 256))
                for n0, ns in N5:
                    for mi, (dst, mw, c0) in enumerate(specs):
                        ps = ps1.tile([128, 480], F32, tag="qkvp")
                        for kc in range(6):
                            nc.tensor.matmul(
                                ps[:mw, :ns],
                                wqkv_s[kc][:, c0 : c0 + mw],
                                xt_s[kc][:, n0 : n0 + ns],
                                start=(kc == 0), stop=(kc == 5),
                            )
                        nc.scalar.activation(
                            dst[:, n0 : n0 + ns], ps[:mw, :ns], AF.Identity,
                            bias=qkvb_s[:mw, mi : mi + 1], scale=1.0,
                        )
                # move heads 2-3 down to base 0/32 (DMA shifts partitions)
                nc.sync.dma_start(qTb[:, :], qTs[64:128, :])
                nc.sync.dma_start(kTb[:, :], kTs[64:128, :])
                if debug:
                    dq = mpool.tile([128, NTOK], F32, tag="dbgq")
                    nc.vector.tensor_copy(dq[:, :], qTs[:, :])
                    nc.sync.dma_start(dbg_q[:, :], dq[:, :])
                    dk = mpool.tile([128, NTOK], F32, tag="dbgk")
                    nc.vector.tensor_copy(dk[:, :], kTs[:, :])
                    nc.sync.dma_start(dbg_k[:, :], dk[:, :])

            # ================= phase 2: attention =================
            if max_phase >= 2:
             with (
                tc.tile_pool(name="att", bufs=2) as apool,
                tc.tile_pool(name="ps2", bufs=2, space="PSUM") as ps2,
                tc.tile_pool(name="ps2z", bufs=2, space="PSUM") as ps2z,
            ):
                for b in range(BL):
                    base = b * T
                    # v = vT.T per s-chunk (all heads at once)
                    for sc, (s0, ss) in enumerate(TB):
                        vtr = ps2.tile([128, DHK], F16, tag="vtr")
                        nc.tensor.transpose(
                            vtr[:ss, :], vT[:, base + s0 : base + s0 + ss],
                            ident[0:DHK, 0:DHK],
                        )
                        nc.scalar.copy(
                            v_s[sc][0:ss, b, :, 0:16],
                            vtr[:ss, :].rearrange("p (h k) -> p h k", h=H),
                        )
                    # FT pass: scores [s, t] with the causal -30000 mask added
                    # into the diag block via an identity-stationary matmul;
                    # exp; then one matmul per (h, sc) computing
                    # [av_h | 0 | Z_h | 0] in a 64-row block; heads 2g / 2g+1
                    # at row bases 0 / 64 of azb[g]
                    azb = [
                        ps2z.tile([128, T], F32, tag=f"azb{g}", name=f"azb{g}")
                        for g in range(2)
                    ]
                    ft = {}
                    for h in range(H):
                        qt = qTs if h < 2 else qTb
                        kt = kTs if h < 2 else kTb
                        q0 = k0 = (h % 2) * 32
                        for sc, (s0, ss) in enumerate(TB):
                            tlen = T - s0
                            weiT = ps2.tile([128, T], F32, tag="weiT")
                            nc.tensor.matmul(
                                weiT[:ss, :tlen],
                                kt[k0 : k0 + HS, base + s0 : base + s0 + ss],
                                qt[q0 : q0 + HS, base + s0 : base + T],
                                start=True, stop=True,
                            )
                            f = apool.tile([128, T], F16, tag=f"ft{h}{sc}")
                            ft[(h, sc)] = f
                            nc.scalar.activation(f[:ss, :tlen], weiT[:ss, :tlen], AF.Exp)
                            nc.gpsimd.affine_select(
                                out=f[:ss, :ss], in_=f[:ss, :ss],
                                compare_op=OP.is_ge, fill=0.0,
                                base=0, channel_multiplier=-1, pattern=[[1, ss]],
                            )
                        for sc, (s0, ss) in enumerate(TB):
                            nc.tensor.matmul(
                                azb[h // 2][64 * (h % 2) : 64 * (h % 2) + 64, s0:T],
                                v_s[sc][0:ss, b, h, :],
                                ft[(h, sc)][0:ss, 0 : T - s0],
                                start=(sc == 0), stop=(sc == 2),
                            )
                    # 1/azb on the scalar table engine (cheap); only the
                    # Z-recip rows are ever read back, so the garbage
                    # av-recip rows (possibly inf) are harmless
                    rz = [
                        apool.tile([128, T], F32, tag=f"rz{g}", name=f"rz{g}")
                        for g in range(2)
                    ]
                    for g in range(2):
                        nc.vector.reciprocal(rz[g][:, :], azb[g][:, :])
                    for h in range(H):
                        g, o = h // 2, 64 * (h % 2)
                        nc.vector.scalar_tensor_tensor(
                            attnT[32 * h : 32 * h + 32, base : base + T],
                            rz[g][o + 32 : o + 64, :], RCLAMP,
                            azb[g][o : o + 32, :],
                            op0=OP.min, op1=OP.mult,
                        )
                if debug:
                    da = apool.tile([128, NTOK], F32, tag="dbga")
                    nc.vector.tensor_copy(da[:, :], attnT[:, :])
                    nc.sync.dma_start(dbg_at[:, :], da[:, :])

            _mid.close()
            _late = ExitStack()
            lpool = _late.enter_context(tc.tile_pool(name="late", bufs=1))
            gp = lpool.tile([128, len(TOK) * E], F32, tag="gp")
            G = lpool.tile([128, len(TOK) * D], F16, tag="G")
            S = lpool.tile([128, len(TOK) * D], F16, tag="S")

            # ================= phase 4: gating =================
            if max_phase >= 4:
             with (
                tc.tile_pool(name="gat", bufs=1) as gpool,
                tc.tile_pool(name="ps4", bufs=2, space="PSUM") as ps4,
                tc.tile_pool(name="ps4t", bufs=1, space="PSUM") as ps4t,
             ):
                NT_ = len(TOK)
                # logits transposed: [8, tok], folded proj+gate stationary
                logT = gpool.tile([8, NTOK], F16, tag="logT")
                for n0, ns in N5:
                    lg = ps4.tile([8, 480], F32, tag="lg")
                    nc.tensor.matmul(
                        lg[:, :ns], wgf_s[:, :], attnT[:, n0 : n0 + ns],
                        start=True, stop=True,
                    )
                    nc.scalar.activation(
                        logT[:, n0 : n0 + ns], lg[:, :ns], AF.Identity,
                        bias=cb_s[:, :], scale=1.0,
                    )
                # transpose back to [tok, 8] in one psum bank
                ltp = ps4t.tile([128, NT_, E], F16, tag="ltp")
                for ti, (t0, ts_) in enumerate(TOK):
                    nc.tensor.transpose(
                        ltp[:ts_, ti, :], logT[:, t0 : t0 + ts_], ident[0:8, 0:8]
                    )
                L3 = gpool.tile([128, NT_, E], F32, tag="L3")
                nc.scalar.copy(L3[:, :, :], ltp[:, :, :])
                work = gpool.tile([128, NT_, E], F32, tag="work")
                nc.vector.tensor_copy(work[:, :, :], L3[:, :, :])
                m = gpool.tile([128, NT_, 4], F32, tag="m")
                eqm = gpool.tile([128, NT_, E], F32, tag="eqm")
                for r in range(TOPK):
                    nc.vector.tensor_reduce(
                        m[:, :, r : r + 1], work[:, :, :], axis=AX.X, op=OP.max,
                        opt_input=False, opt_output=False,
                    )
                    if r < TOPK - 1:
                        nc.vector.tensor_tensor(
                            eqm[:, :, :], work[:, :, :],
                            m[:, :, r : r + 1].broadcast_to((128, NT_, E)),
                            op=OP.is_equal,
                        )
                        nc.vector.scalar_tensor_tensor(
                            work[:, :, :], eqm[:, :, :], NEG, work[:, :, :],
                            op0=OP.mult, op1=OP.add,
                        )
                sel = gpool.tile([128, NT_, E], F32, tag="sel")
                nc.vector.tensor_tensor(
                    sel[:, :, :], L3[:, :, :],
                    m[:, :, 3:4].broadcast_to((128, NT_, E)), op=OP.is_ge,
                )
                el = gpool.tile([128, NT_, E], F32, tag="el")
                nc.scalar.activation(el[:, :, :], L3[:, :, :], AF.Exp)
                elm = gpool.tile([128, NT_, E], F32, tag="elm")
                nc.vector.tensor_tensor(elm[:, :, :], el[:, :, :], sel[:, :, :], op=OP.mult)
                zg = gpool.tile([128, NT_, 1], F32, tag="zg")
                nc.vector.tensor_reduce(
                    zg[:, :, :], elm[:, :, :], axis=AX.X, op=OP.add,
                    opt_input=False, opt_output=False,
                )
                rzg = gpool.tile([128, NT_, 1], F32, tag="rzg")
                nc.vector.reciprocal(rzg[:, :, :], zg[:, :, :])
                nc.vector.tensor_tensor(
                    gp[:, :].rearrange("p (t e) -> p t e", e=E), elm[:, :, :],
                    rzg[:, :, :].broadcast_to((128, NT_, E)),
                    op=OP.mult,
                )
            if debug and max_phase >= 4:
                nc.sync.dma_start(dbg_gp[:, :], gp[:, :])

            # ================= phase 5: experts =================
            if max_phase >= 5:
             with (
                tc.tile_pool(name="outp", bufs=4) as opool,
                tc.tile_pool(name="wexp", bufs=1) as wpool,
                tc.tile_pool(name="ht", bufs=2) as hpool,
                tc.tile_pool(name="ps5", bufs=4, space="PSUM") as ps5h,
                tc.tile_pool(name="ps5b", bufs=4, space="PSUM") as ps5e,
            ):
                for e in range(E // 2, E):
                    t = wpool.tile([128, DH], F16, tag=f"w1_{e}", name=f"w1_{e}")
                    nc.gpsimd.dma_start(t[:, :], w1_d[e, :, :])
                    w1_all.append(t)
                    w2t = []
                    for mc, (h0, hs) in enumerate(MCH):
                        t = wpool.tile([hs, D], F16, tag=f"w2_{e}_{mc}", name=f"w2_{e}_{mc}")
                        nc.gpsimd.dma_start(t[:, :], w2_d[e, h0 : h0 + hs, :])
                        w2t.append(t)
                    w2_all.append(w2t)
                    if b2_nonzero:
                        t = wpool.tile([1, D], F16, tag=f"b2row{e}", name=f"b2row{e}")
                        nc.gpsimd.dma_start(t[:, :], b2_d[e, :, :])
                        b2_all.append(t)
                ones = None
                if b2_nonzero:
                    ones = cpool.tile([1, NTOK], F16, tag="ones")
                    nc.gpsimd.memset(ones[:, :], 1.0)
                for e in range(E):
                    w1t = w1_all[e]
                    w2t = w2_all[e]
                    b2row = b2_all[e] if b2_nonzero else None
                    ht = [
                        hpool.tile([hs, NTOK], F16, tag=f"ht{mc}", name=f"ht{mc}")
                        for mc, (h0, hs) in enumerate(MCH)
                    ]
                    for n, (n0, ns) in enumerate(N5):
                        for mc, (h0, hs) in enumerate(MCH):
                            hp = ps5h.tile([128, 480], F32, tag="hp")
                            nc.tensor.matmul(
                                hp[:hs, :ns], w1t[:, h0 : h0 + hs],
                                attnT[:, n0 : n0 + ns],
                                start=True, stop=True,
                            )
                            nc.scalar.activation(
                                ht[mc][:, n0 : n0 + ns], hp[:hs, :ns], AF.Relu,
                                bias=b1_s[:hs, e * 5 + mc : e * 5 + mc + 1], scale=1.0,
                            )
                    for ti, (t0, ts_) in enumerate(TOK):
                        eo = ps5e.tile([128, D], F32, tag="eo")
                        for mc in range(5):
                            nc.tensor.matmul(
                                eo[:ts_, :], ht[mc][:, t0 : t0 + ts_], w2t[mc][:, :],
                                start=(mc == 0), stop=(mc == 4 and not b2_nonzero),
                            )
                        if b2_nonzero:
                            nc.tensor.matmul(
                                eo[:ts_, :], ones[:, t0 : t0 + ts_], b2row[:, :],
                                start=False, stop=True,
                            )
                        gsc = gp[:ts_, ti * E + e : ti * E + e + 1]
                        gsl = G[:ts_, ti * D : (ti + 1) * D]
                        ssl = S[:ts_, ti * D : (ti + 1) * D]
                        if e == 0:
                            nc.vector.tensor_scalar_mul(gsl, eo[:ts_, :], gsc)
                            nc.scalar.copy(ssl, eo[:ts_, :])
                        else:
                            nc.vector.scalar_tensor_tensor(
                                gsl, eo[:ts_, :], gsc, gsl, op0=OP.mult, op1=OP.add
                            )
                            nc.vector.tensor_tensor(ssl, eo[:ts_, :], ssl, op=OP.add)
                        if e == E - 1:
                            o = opool.tile([128, D], F32, tag="o")
                            nc.vector.tensor_tensor(o[:ts_, :], gsl, ssl, op=OP.mult)
                            nc.sync.dma_start(out_d[t0 : t0 + ts_, :], o[:ts_, :])

            _late.close()

    _split_waits(nc)
    return nc


_CACHE = {}
LAST_RESULT = None


def _get_module(b2_nonzero: bool, debug: bool = False, max_phase: int = 9):
    key = (b2_nonzero, debug, max_phase)
    if key not in _CACHE:
        _CACHE[key] = _build_module(b2_nonzero, debug=debug, max_phase=max_phase)
    return _CACHE[key]


def _prep_inputs(x, wh_bias, wh_W, Wq, Wk, Wv, proj_W, proj_b,
                 exp_W1, exp_b1, exp_W2, exp_b2, w_gate):
    # fold whiten into qkv; fold attention scale into q
    scale = float(D) ** -0.5
    Wqf = (Wq.reshape(DHK, D) @ wh_W) * scale          # [64, 768]
    Wkf = Wk.reshape(DHK, D) @ wh_W
    Wvf = Wv.reshape(DHK, D) @ wh_W

    def stripe(w):                                     # [64, 768] -> [128, 768]
        out = np.zeros((128, DIN), np.float32)
        for h in range(H):
            out[h * 32 : h * 32 + HS] = w[h * HS : (h + 1) * HS]
        return out

    def stripe_b(v):                                   # [64] -> [128]
        out = np.zeros(128, np.float32)
        for h in range(H):
            out[h * 32 : h * 32 + HS] = v[h * HS : (h + 1) * HS]
        return out

    wqkv = np.concatenate([stripe(Wqf), stripe(Wkf), Wvf], 0)   # [320, 768]
    bq = -(Wqf @ wh_bias)
    bk = -(Wkf @ wh_bias)
    bv = -(Wvf @ wh_bias)
    qkvb = np.stack([stripe_b(bq), stripe_b(bk), np.pad(bv, (0, 64))], 1)  # [128, 3]

    # head-striped proj weights [128, D]: rows 32h:32h+16 = head h, zeros
    # elsewhere (attnT garbage rows are the constant Z/Z = 1 there)
    projwS = np.zeros((128, D), np.float64)
    for h in range(H):
        projwS[32 * h : 32 * h + HS] = proj_W[:, h * HS : (h + 1) * HS].T

    # fold proj into the gate and expert W1 weights (fp64 host math)
    wgf = projwS @ w_gate.astype(np.float64)               # [128, E]
    cb = proj_b.astype(np.float64) @ w_gate.astype(np.float64)   # [E]
    w1f = np.einsum(
        "pd,ehd->eph", projwS, exp_W1.astype(np.float64)
    )                                                      # [E, 128, 600]
    b1f = exp_W1.astype(np.float64) @ proj_b.astype(np.float64) + exp_b1  # [E, 600]

    # b1 bias slices [128, E*5]: column e*5+mc holds b1f[e, h0:h0+hs]
    b1s = np.zeros((128, E * 5), np.float32)
    for e in range(E):
        for mc, (h0, hs) in enumerate(MCH):
            b1s[:hs, e * 5 + mc] = b1f[e, h0 : h0 + hs]

    f16 = np.float16
    common = {
        "wqkv": np.ascontiguousarray(wqkv.T).astype(f16),
        "qkvb": np.ascontiguousarray(qkvb).astype(np.float32),
        "wgf": np.ascontiguousarray(wgf).astype(f16),
        "cb": np.ascontiguousarray(cb[:, None]).astype(np.float32),
        "idm": np.eye(128, dtype=f16),
        "w1t": np.ascontiguousarray(w1f).astype(f16),
        "w2t": np.ascontiguousarray(exp_W2.transpose(0, 2, 1)).astype(f16),
        "b1s": b1s.astype(np.float32),
        "b2s": np.ascontiguousarray(exp_b2[:, None, :]).astype(f16),
    }
    in_maps = []
    for c in range(NCORES):
        xc = x[c * BL : (c + 1) * BL]                  # [8, 300, 768]
        xt = np.ascontiguousarray(
            xc.transpose(2, 0, 1).reshape(DIN, NTOK)
        ).astype(f16)
        in_maps.append({**common, "xt": xt})
    return in_maps


def kernel(x, wh_bias, wh_W, Wq, Wk, Wv, proj_W, proj_b,
           exp_W1, exp_b1, exp_W2, exp_b2, w_gate,
           debug=False, max_phase=9):
    global LAST_RESULT
    x = np.asarray(x, np.float32)
    wh_bias = np.asarray(wh_bias, np.float32)
    wh_W = np.asarray(wh_W, np.float32)
    Wq, Wk, Wv = (np.asarray(w, np.float32) for w in (Wq, Wk, Wv))
    proj_W = np.asarray(proj_W, np.float32)
    proj_b = np.asarray(proj_b, np.float32)
    exp_W1 = np.asarray(exp_W1, np.float32)
    exp_b1 = np.asarray(exp_b1, np.float32)
    exp_W2 = np.asarray(exp_W2, np.float32)
    exp_b2 = np.asarray(exp_b2, np.float32)
    w_gate = np.asarray(w_gate, np.float32)

    b2_nonzero = bool(np.any(exp_b2))
    in_maps = _prep_inputs(x, wh_bias, wh_W, Wq, Wk, Wv, proj_W, proj_b,
                           exp_W1, exp_b1, exp_W2, exp_b2, w_gate)

    nc = _get_module(b2_nonzero, debug=debug, max_phase=max_phase)
    for alloc in nc.m.functions[0].allocations:
        if isinstance(alloc, mybir.MemoryLocationSet) and alloc.kind == "ExternalInput":
            nm = alloc.memorylocations[0].name
            if nm not in in_maps[0]:
                continue  # partition_id etc., supplied by the runner
            got = in_maps[0][nm]
            assert tuple(got.shape) == tuple(alloc.tensor_shape), (
                nm, got.shape, alloc.tensor_shape)
            assert got.dtype == mybir.dt.np(alloc.dtype), (nm, got.dtype)
    res = run_bass_kernel_spmd(nc, in_maps, core_ids=list(range(NCORES)))
    LAST_RESULT = res
    out = np.stack([r["out"] for r in res.results])    # [8, 2400, 300]
    return out.reshape(B, T, D)


# revision 38
# speedup vs baseline: 1.0879x; 1.0148x over previous
"""Trainium2 Bass kernel for nn_MoEAdaptorLayer (whiten -> causal MHA -> proj
-> noisy-top-k gating (eval) -> 8 dense experts -> gated mixture * expert sum).

Sharding: data-parallel over batch. 64 batches -> 8 per core, params replicated.

v2 redesign vs baseline:
- fp16 everywhere on the matmul path (fp32 PSUM accumulation); halves DMA and
  removes the fp32r small-free-dim PE penalties.
- qkv: x shipped as one [768, 2400] fp16 tensor per core (6 big DMAs), matmuls
  in [128, 480] chunks.
- attention: single-orientation flash-style pass. Scores computed [s, t] only;
  exp on scalar; causal mask by multiplying the diagonal blocks with a
  triangular constant (split vector/gpsimd). AV and the softmax denominator
  come from ONE matmul per (head, s-chunk): stationary [v_h | ones] (32 cols)
  so the psum holds [av_h (16 rows) | Z_h replicated (16 rows)] per 32-stripe.
  One reciprocal + one fused min-mult STT per batch produce attnT directly in
  head-striped [128, tok] layout (no Z-pass, no output transposes).
- proj: stationary is zero-padded to the 32-stripe layout; 15 fp16 matmuls.
- gating: logits computed transposed ([8, tok], weight-stationary, 15 matmuls
  instead of 57 LDW-bound ones), PE-transposed back in 19 tiny transposes.
- experts: h chunked {128,128,128,128,88} (M=128-aligned LDWEIGHTS is ~2x
  faster than M=120), fp16 weights/activations.
"""

from contextlib import ExitStack

import numpy as np

import concourse.bass as bass
import concourse.tile as tile
import concourse.mybir as mybir
from concourse.bass_utils import run_bass_kernel_spmd

F16 = mybir.dt.float16
F32 = mybir.dt.float32
AX = mybir.AxisListType
OP = mybir.AluOpType
AF = mybir.ActivationFunctionType

B, T, DIN, D, E, H, HS = 64, 300, 768, 300, 8, 4, 16
NCORES = 8
BL = B // NCORES          # 8 batches per core
NTOK = BL * T             # 2400 tokens per core
DHK = H * HS              # 64
DH = 2 * D                # 600
TOPK = E // 2

K768 = [(i * 128, 128) for i in range(6)]
DC = [(0, 128), (128, 128), (256, 44)]          # 300 = 128+128+44
MCH = [(0, 128), (128, 128), (256, 128), (384, 128), (512, 88)]  # 600
TB = [(0, 128), (128, 128), (256, 44)]          # 300 tokens per batch
TOK = [(i * 128, 128) for i in range(18)] + [(2304, 96)]   # 2400 tokens
N5 = [(i * 480, 480) for i in range(5)]         # 2400 free-dim chunks
NEG = -1.0e30
RCLAMP = 16.0   # cap on 1/Z; keeps garbage stripe rows finite in fp16

_MAX_DRAIN_WAITS = 1
_WAIT_LIMIT = 1


def _split_waits(nc):
    """Walrus in this build caps sync waits per instruction; hoist excess
    waits onto same-engine NOPs inserted just before the instruction."""
    n = 0
    for f in nc.m.functions:
        for blk in f.blocks:
            insts = blk.instructions
            out = []
            changed = False
            for inst in insts:
                si = inst.sync_info
                waits = list(si.on_wait or []) if si is not None else []
                if len(waits) > _WAIT_LIMIT:
                    head, tail = waits[:-_WAIT_LIMIT], waits[-_WAIT_LIMIT:]
                    for i in range(0, len(head), _WAIT_LIMIT):
                        n += 1
                        nop = mybir.InstNoOp(name=f"waitnop{n}", ins=[], outs=[])
                        nop.engine = inst.engine
                        nop.sync_info = mybir.SyncInfo(
                            on_wait=head[i : i + _WAIT_LIMIT], on_update=[]
                        )
                        out.append(nop)
                    si.on_wait = tail
                    inst.sync_info = si
                    changed = True
                out.append(inst)
            if changed:
                blk.instructions = out


def _install_drain_patch():
    """This walrus build rejects CTRL instructions with more than a few sync
    waits; Tile's tail drain waits on every engine/DMA semaphore at once.
    Split the waits across a chain of single-wait drains."""
    if getattr(tile.TileContext, "_drain_patched", False):
        return

    def _patched(self, tick_clock, wait_clock):
        nc = self.nc
        drain_inst = nc.sync.drain()
        wait_clock.add_sem_waits(
            drain_inst.ins, tile.ScopedClock({None: tick_clock.global_clock})
        )
        ri = drain_inst.ins
        si = ri.sync_info
        waits = list(si.on_wait or []) if si is not None else []
        if len(waits) > _MAX_DRAIN_WAITS:
            si.on_wait = waits[:_MAX_DRAIN_WAITS]
            ri.sync_info = si
            for i in range(_MAX_DRAIN_WAITS, len(waits), _MAX_DRAIN_WAITS):
                d2 = nc.sync.drain()
                d2.ins.sync_info = mybir.SyncInfo(
                    on_wait=waits[i : i + _MAX_DRAIN_WAITS], on_update=[]
                )
        nc.all_engine_barrier()
        assert self.sems is not None
        popped = nc._tile_sem_poison_stack.pop()
        assert popped is self._sem_poison
        nc.clear_and_free_semaphores(list(self.sems.allocated().values()))
        nc.all_engine_barrier()

    tile.TileContext._drain_and_barrier = _patched
    tile.TileContext._drain_patched = True


def _build_module(b2_nonzero: bool, debug: bool = False, max_phase: int = 9):
    _install_drain_patch()
    nc = bass.Bass("TRN2", target_bir_lowering=False, debug=False)

    # ---- DRAM I/O ----
    xt_d = nc.dram_tensor("xt", [DIN, NTOK], F16, kind="ExternalInput")
    wqkv_d = nc.dram_tensor("wqkv", [DIN, 320], F16, kind="ExternalInput")
    qkvb_d = nc.dram_tensor("qkvb", [128, 3], F32, kind="ExternalInput")
    wgf_d = nc.dram_tensor("wgf", [128, E], F16, kind="ExternalInput")
    cb_d = nc.dram_tensor("cb", [E, 1], F32, kind="ExternalInput")
    w1_d = nc.dram_tensor("w1t", [E, 128, DH], F16, kind="ExternalInput")
    w2_d = nc.dram_tensor("w2t", [E, DH, D], F16, kind="ExternalInput")
    b1_d = nc.dram_tensor("b1s", [128, E * 5], F32, kind="ExternalInput")
    b2_d = nc.dram_tensor("b2s", [E, 1, D], F16, kind="ExternalInput")
    idm_d = nc.dram_tensor("idm", [128, 128], F16, kind="ExternalInput")
    out_d = nc.dram_tensor("out", [NTOK, D], F32, kind="ExternalOutput")
    if debug:
        dbg_q = nc.dram_tensor("dbg_q", [128, NTOK], F32, kind="ExternalOutput")
        dbg_k = nc.dram_tensor("dbg_k", [128, NTOK], F32, kind="ExternalOutput")
        dbg_at = nc.dram_tensor("dbg_at", [128, NTOK], F32, kind="ExternalOutput")
        dbg_xa = nc.dram_tensor("dbg_xa", [D, NTOK], F32, kind="ExternalOutput")
        dbg_gp = nc.dram_tensor("dbg_gp", [128, len(TOK) * E], F32, kind="ExternalOutput")

    with tile.TileContext(nc) as tc:
        with (
            tc.tile_pool(name="const", bufs=1) as cpool,
            tc.tile_pool(name="big", bufs=1) as bpool,
        ):
            # ---- persistent constants ----
            ident = cpool.tile([128, 128], F16)
            wqkv_s = []
            for kc, (k0, ks) in enumerate(K768):
                t = cpool.tile([128, 320], F16, tag=f"wqkv{kc}")
                nc.sync.dma_start(t[:, :], wqkv_d[k0 : k0 + ks, :])
                wqkv_s.append(t)
            qkvb_s = cpool.tile([128, 3], F32, tag="qkvbs")
            nc.sync.dma_start(qkvb_s[:, :], qkvb_d[:, :])
            wgf_s = cpool.tile([128, E], F16, tag="wgf")
            nc.sync.dma_start(wgf_s[:, :], wgf_d[:, :])
            cb_s = cpool.tile([E, 1], F32, tag="cb")
            nc.sync.dma_start(cb_s[:, :], cb_d[:, :])
            b1_s = cpool.tile([128, E * 5], F32, tag="b1s")
            nc.sync.dma_start(b1_s[:, :], b1_d[:, :])
            nc.sync.dma_start(ident[:, :], idm_d[:, :])

            # x input on the gpsimd queue ahead of the expert weights
            xt_s = [
                bpool.tile([128, NTOK], F16, tag=f"xt{kc}", name=f"xt{kc}")
                for kc in range(6)
            ]
            for kc, (k0, ks) in enumerate(K768):
                nc.gpsimd.dma_start(xt_s[kc][:, :], xt_d[k0 : k0 + ks, :])

            # prefetch experts 0-3 weights up front on the gpsimd DMA queue
            w1_all, w2_all, b2_all = [], [], []
            for e in range(E // 2):
                t = bpool.tile([128, DH], F16, tag=f"w1_{e}", name=f"w1_{e}")
                nc.gpsimd.dma_start(t[:, :], w1_d[e, :, :])
                w1_all.append(t)
                w2t = []
                for mc, (h0, hs) in enumerate(MCH):
                    t = bpool.tile([hs, D], F16, tag=f"w2_{e}_{mc}", name=f"w2_{e}_{mc}")
                    nc.gpsimd.dma_start(t[:, :], w2_d[e, h0 : h0 + hs, :])
                    w2t.append(t)
                w2_all.append(w2t)
                if b2_nonzero:
                    t = bpool.tile([1, D], F16, tag=f"b2row{e}", name=f"b2row{e}")
                    nc.gpsimd.dma_start(t[:, :], b2_d[e, :, :])
                    b2_all.append(t)

            # ---- activations ----
            _mid = ExitStack()
            mpool = _mid.enter_context(tc.tile_pool(name="mid", bufs=1))
            # heads striped at 32-partition bases (rows 32h:32h+16 per head)
            qTs = mpool.tile([128, NTOK], F16, tag="qTs")
            kTs = mpool.tile([128, NTOK], F16, tag="kTs")
            qTb = mpool.tile([DHK, NTOK], F16, tag="qTb")
            kTb = mpool.tile([DHK, NTOK], F16, tag="kTb")
            vT = mpool.tile([DHK, NTOK], F16, tag="vT")
            # v_s[sc]: [ss, b, h, 64] = [v_h(16) | 0(16) | ones(16) | 0(16)];
            # the av/Z layout this produces keeps every vector-op partition
            # base 32-aligned
            v_s = [
                mpool.tile([128, BL, H, 64], F16, tag=f"v{sc}", name=f"v{sc}")
                for sc in range(3)
            ]
            # attnT [128, NTOK]: rows 32h:32h+16 = head h, other rows are
            # the constant 1.0 (Z/Z); folded weights are zero there
            attnT = bpool.tile([128, NTOK], F16, tag="attnT")
            # v_s constant columns [v16 | ones48]: one matmul per (h, sc)
            # produces [av (16 rows) | Z replicated (48 rows)]
            for sc in range(3):
                nc.gpsimd.memset(v_s[sc][:, :, :, 16:64], 1.0)

            # ================= phase 1: qkv =================
            with (
                tc.tile_pool(name="ps1", bufs=4, space="PSUM") as ps1,
            ):

                specs = ((qTs, 128, 0), (kTs, 128, 128), (vT, DHK, 256))
                for n0, ns in N5:
                    for mi, (dst, mw, c0) in enumerate(specs):
                        ps = ps1.tile([128, 480], F32, tag="qkvp")
                        for kc in range(6):
                            nc.tensor.matmul(
                                ps[:mw, :ns],
                                wqkv_s[kc][:, c0 : c0 + mw],
                                xt_s[kc][:, n0 : n0 + ns],
                                start=(kc == 0), stop=(kc == 5),
                            )
                        nc.scalar.activation(
                            dst[:, n0 : n0 + ns], ps[:mw, :ns], AF.Identity,
                            bias=qkvb_s[:mw, mi : mi + 1], scale=1.0,
                        )
                # move heads 2-3 down to base 0/32 (DMA shifts partitions)
                nc.sync.dma_start(qTb[:, :], qTs[64:128, :])
                nc.sync.dma_start(kTb[:, :], kTs[64:128, :])
                if debug:
                    dq = mpool.tile([128, NTOK], F32, tag="dbgq")
                    nc.vector.tensor_copy(dq[:, :], qTs[:, :])
                    nc.sync.dma_start(dbg_q[:, :], dq[:, :])
                    dk = mpool.tile([128, NTOK], F32, tag="dbgk")
                    nc.vector.tensor_copy(dk[:, :], kTs[:, :])
                    nc.sync.dma_start(dbg_k[:, :], dk[:, :])

            # ================= phase 2: attention =================
            if max_phase >= 2:
             with (
                tc.tile_pool(name="att", bufs=2) as apool,
                tc.tile_pool(name="ps2", bufs=2, space="PSUM") as ps2,
                tc.tile_pool(name="ps2z", bufs=2, space="PSUM") as ps2z,
            ):
                for b in range(BL):
                    base = b * T
                    # v = vT.T per s-chunk (all heads at once)
                    for sc, (s0, ss) in enumerate(TB):
                        vtr = ps2.tile([128, DHK], F16, tag="vtr")
                        nc.tensor.transpose(
                            vtr[:ss, :], vT[:, base + s0 : base + s0 + ss],
                            ident[0:DHK, 0:DHK],
                        )
                        nc.scalar.copy(
                            v_s[sc][0:ss, b, :, 0:16],
                            vtr[:ss, :].rearrange("p (h k) -> p h k", h=H),
                        )
                    # FT pass: scores [s, t] with the causal -30000 mask added
                    # into the diag block via an identity-stationary matmul;
                    # exp; then one matmul per (h, sc) computing
                    # [av_h | 0 | Z_h | 0] in a 64-row block; heads 2g / 2g+1
                    # at row bases 0 / 64 of azb[g]
                    azb = [
                        ps2z.tile([128, T], F32, tag=f"azb{g}", name=f"azb{g}")
                        for g in range(2)
                    ]
                    ft = {}
                    for h in range(H):
                        qt = qTs if h < 2 else qTb
                        kt = kTs if h < 2 else kTb
                        q0 = k0 = (h % 2) * 32
                        for sc, (s0, ss) in enumerate(TB):
                            tlen = T - s0
                            weiT = ps2.tile([128, T], F32, tag="weiT")
                            nc.tensor.matmul(
                                weiT[:ss, :tlen],
                                kt[k0 : k0 + HS, base + s0 : base + s0 + ss],
                                qt[q0 : q0 + HS, base + s0 : base + T],
                                start=True, stop=True,
                            )
                            f = apool.tile([128, T], F16, tag=f"ft{h}{sc}")
                            ft[(h, sc)] = f
                            nc.scalar.activation(f[:ss, :tlen], weiT[:ss, :tlen], AF.Exp)
                            nc.gpsimd.affine_select(
                                out=f[:ss, :ss], in_=f[:ss, :ss],
                                compare_op=OP.is_ge, fill=0.0,
                                base=0, channel_multiplier=-1, pattern=[[1, ss]],
                            )
                        for sc, (s0, ss) in enumerate(TB):
                            nc.tensor.matmul(
                                azb[h // 2][64 * (h % 2) : 64 * (h % 2) + 64, s0:T],
                                v_s[sc][0:ss, b, h, :],
                                ft[(h, sc)][0:ss, 0 : T - s0],
                                start=(sc == 0), stop=(sc == 2),
                            )
                    # 1/azb on the scalar table engine (cheap); only the
                    # Z-recip rows are ever read back, so the garbage
                    # av-recip rows (possibly inf) are harmless
                    rz = [
                        apool.tile([128, T], F32, tag=f"rz{g}", name=f"rz{g}")
                        for g in range(2)
                    ]
                    for g in range(2):
                        nc.vector.reciprocal(rz[g][:, :], azb[g][:, :])
                    for h in range(H):
                        g, o = h // 2, 64 * (h % 2)
                        nc.vector.scalar_tensor_tensor(
                            attnT[32 * h : 32 * h + 32, base : base + T],
                            rz[g][o + 32 : o + 64, :], RCLAMP,
                            azb[g][o : o + 32, :],
                            op0=OP.min, op1=OP.mult,
                        )
                if debug:
                    da = apool.tile([128, NTOK], F32, tag="dbga")
                    nc.vector.tensor_copy(da[:, :], attnT[:, :])
                    nc.sync.dma_start(dbg_at[:, :], da[:, :])

            _mid.close()
            _late = ExitStack()
            lpool = _late.enter_context(tc.tile_pool(name="late", bufs=1))
            gp = lpool.tile([128, len(TOK) * E], F32, tag="gp")
            G = lpool.tile([128, len(TOK) * D], F32, tag="G")
            S = lpool.tile([128, len(TOK) * D], F32, tag="S")

            # ================= phase 4: gating =================
            if max_phase >= 4:
             with (
                tc.tile_pool(name="gat", bufs=1) as gpool,
                tc.tile_pool(name="ps4", bufs=2, space="PSUM") as ps4,
                tc.tile_pool(name="ps4t", bufs=1, space="PSUM") as ps4t,
             ):
                NT_ = len(TOK)
                # logits transposed: [8, tok], folded proj+gate stationary
                logT = gpool.tile([8, NTOK], F16, tag="logT")
                for n0, ns in N5:
                    lg = ps4.tile([8, 480], F32, tag="lg")
                    nc.tensor.matmul(
                        lg[:, :ns], wgf_s[:, :], attnT[:, n0 : n0 + ns],
                        start=True, stop=True,
                    )
                    nc.scalar.activation(
                        logT[:, n0 : n0 + ns], lg[:, :ns], AF.Identity,
                        bias=cb_s[:, :], scale=1.0,
                    )
                # transpose back to [tok, 8] in one psum bank
                ltp = ps4t.tile([128, NT_, E], F16, tag="ltp")
                for ti, (t0, ts_) in enumerate(TOK):
                    nc.tensor.transpose(
                        ltp[:ts_, ti, :], logT[:, t0 : t0 + ts_], ident[0:8, 0:8]
                    )
                L3 = gpool.tile([128, NT_, E], F32, tag="L3")
                nc.scalar.copy(L3[:, :, :], ltp[:, :, :])
                work = gpool.tile([128, NT_, E], F32, tag="work")
                nc.vector.tensor_copy(work[:, :, :], L3[:, :, :])
                m = gpool.tile([128, NT_, 4], F32, tag="m")
                eqm = gpool.tile([128, NT_, E], F32, tag="eqm")
                for r in range(TOPK):
                    nc.vector.tensor_reduce(
                        m[:, :, r : r + 1], work[:, :, :], axis=AX.X, op=OP.max,
                        opt_input=False, opt_output=False,
                    )
                    if r < TOPK - 1:
                        nc.vector.tensor_tensor(
                            eqm[:, :, :], work[:, :, :],
                            m[:, :, r : r + 1].broadcast_to((128, NT_, E)),
                            op=OP.is_equal,
                        )
                        nc.vector.scalar_tensor_tensor(
                            work[:, :, :], eqm[:, :, :], NEG, work[:, :, :],
                            op0=OP.mult, op1=OP.add,
                        )
                sel = gpool.tile([128, NT_, E], F32, tag="sel")
                nc.vector.tensor_tensor(
                    sel[:, :, :], L3[:, :, :],
                    m[:, :, 3:4].broadcast_to((128, NT_, E)), op=OP.is_ge,
                )
                el = gpool.tile([128, NT_, E], F32, tag="el")
                nc.scalar.activation(el[:, :, :], L3[:, :, :], AF.Exp)
                elm = gpool.tile([128, NT_, E], F32, tag="elm")
                nc.vector.tensor_tensor(elm[:, :, :], el[:, :, :], sel[:, :, :], op=OP.mult)
                zg = gpool.tile([128, NT_, 1], F32, tag="zg")
                nc.vector.tensor_reduce(
                    zg[:, :, :], elm[:, :, :], axis=AX.X, op=OP.add,
                    opt_input=False, opt_output=False,
                )
                rzg = gpool.tile([128, NT_, 1], F32, tag="rzg")
                nc.vector.reciprocal(rzg[:, :, :], zg[:, :, :])
                nc.vector.tensor_tensor(
                    gp[:, :].rearrange("p (t e) -> p t e", e=E), elm[:, :, :],
                    rzg[:, :, :].broadcast_to((128, NT_, E)),
                    op=OP.mult,
                )
            if debug and max_phase >= 4:
                nc.sync.dma_start(dbg_gp[:, :], gp[:, :])

            # ================= phase 5: experts =================
            if max_phase >= 5:
             with (
                tc.tile_pool(name="outp", bufs=4) as opool,
                tc.tile_pool(name="wexp", bufs=1) as wpool,
                tc.tile_pool(name="ht", bufs=2) as hpool,
                tc.tile_pool(name="ps5", bufs=4, space="PSUM") as ps5h,
                tc.tile_pool(name="ps5b", bufs=4, space="PSUM") as ps5e,
            ):
                for e in range(E // 2, E):
                    t = wpool.tile([128, DH], F16, tag=f"w1_{e}", name=f"w1_{e}")
                    nc.gpsimd.dma_start(t[:, :], w1_d[e, :, :])
                    w1_all.append(t)
                    w2t = []
                    for mc, (h0, hs) in enumerate(MCH):
                        t = wpool.tile([hs, D], F16, tag=f"w2_{e}_{mc}", name=f"w2_{e}_{mc}")
                        nc.gpsimd.dma_start(t[:, :], w2_d[e, h0 : h0 + hs, :])
                        w2t.append(t)
                    w2_all.append(w2t)
                    if b2_nonzero:
                        t = wpool.tile([1, D], F16, tag=f"b2row{e}", name=f"b2row{e}")
                        nc.gpsimd.dma_start(t[:, :], b2_d[e, :, :])
                        b2_all.append(t)
                ones = None
                if b2_nonzero:
                    ones = cpool.tile([1, NTOK], F16, tag="ones")
                    nc.gpsimd.memset(ones[:, :], 1.0)
                for e in range(E):
                    w1t = w1_all[e]
                    w2t = w2_all[e]
                    b2row = b2_all[e] if b2_nonzero else None
                    ht = [
                        hpool.tile([hs, NTOK], F16, tag=f"ht{mc}", name=f"ht{mc}")
                        for mc, (h0, hs) in enumerate(MCH)
                    ]
                    for n, (n0, ns) in enumerate(N5):
                        for mc, (h0, hs) in enumerate(MCH):
                            hp = ps5h.tile([128, 480], F32, tag="hp")
                            nc.tensor.matmul(
                                hp[:hs, :ns], w1t[:, h0 : h0 + hs],
                                attnT[:, n0 : n0 + ns],
                                start=True, stop=True,
                            )
                            nc.scalar.activation(
                                ht[mc][:, n0 : n0 + ns], hp[:hs, :ns], AF.Relu,
                                bias=b1_s[:hs, e * 5 + mc : e * 5 + mc + 1], scale=1.0,
                            )
                    for ti, (t0, ts_) in enumerate(TOK):
                        eo = ps5e.tile([128, D], F32, tag="eo")
                        for mc in range(5):
                            nc.tensor.matmul(
                                eo[:ts_, :], ht[mc][:, t0 : t0 + ts_], w2t[mc][:, :],
                                start=(mc == 0), stop=(mc == 4 and not b2_nonzero),
                            )
                        if b2_nonzero:
                            nc.tensor.matmul(
                                eo[:ts_, :], ones[:, t0 : t0 + ts_], b2row[:, :],
                                start=False, stop=True,
                            )
                        gsc = gp[:ts_, ti * E + e : ti * E + e + 1]
                        gsl = G[:ts_, ti * D : (ti + 1) * D]
                        ssl = S[:ts_, ti * D : (ti + 1) * D]
                        if e == 0:
                            nc.vector.tensor_scalar_mul(gsl, eo[:ts_, :], gsc)
                            nc.scalar.copy(ssl, eo[:ts_, :])
                        else:
                            nc.vector.scalar_tensor_tensor(
                                gsl, eo[:ts_, :], gsc, gsl, op0=OP.mult, op1=OP.add
                            )
                            nc.vector.tensor_tensor(ssl, eo[:ts_, :], ssl, op=OP.add)
                        if e == E - 1:
                            o = opool.tile([128, D], F32, tag="o")
                            nc.gpsimd.tensor_tensor(o[:ts_, :], gsl, ssl, op=OP.mult)
                            nc.sync.dma_start(out_d[t0 : t0 + ts_, :], o[:ts_, :])

            _late.close()

    _split_waits(nc)
    return nc


_CACHE = {}
LAST_RESULT = None


def _get_module(b2_nonzero: bool, debug: bool = False, max_phase: int = 9):
    key = (b2_nonzero, debug, max_phase)
    if key not in _CACHE:
        _CACHE[key] = _build_module(b2_nonzero, debug=debug, max_phase=max_phase)
    return _CACHE[key]


def _prep_inputs(x, wh_bias, wh_W, Wq, Wk, Wv, proj_W, proj_b,
                 exp_W1, exp_b1, exp_W2, exp_b2, w_gate):
    # fold whiten into qkv; fold attention scale into q
    scale = float(D) ** -0.5
    Wqf = (Wq.reshape(DHK, D) @ wh_W) * scale          # [64, 768]
    Wkf = Wk.reshape(DHK, D) @ wh_W
    Wvf = Wv.reshape(DHK, D) @ wh_W

    def stripe(w):                                     # [64, 768] -> [128, 768]
        out = np.zeros((128, DIN), np.float32)
        for h in range(H):
            out[h * 32 : h * 32 + HS] = w[h * HS : (h + 1) * HS]
        return out

    def stripe_b(v):                                   # [64] -> [128]
        out = np.zeros(128, np.float32)
        for h in range(H):
            out[h * 32 : h * 32 + HS] = v[h * HS : (h + 1) * HS]
        return out

    wqkv = np.concatenate([stripe(Wqf), stripe(Wkf), Wvf], 0)   # [320, 768]
    bq = -(Wqf @ wh_bias)
    bk = -(Wkf @ wh_bias)
    bv = -(Wvf @ wh_bias)
    qkvb = np.stack([stripe_b(bq), stripe_b(bk), np.pad(bv, (0, 64))], 1)  # [128, 3]

    # head-striped proj weights [128, D]: rows 32h:32h+16 = head h, zeros
    # elsewhere (attnT garbage rows are the constant Z/Z = 1 there)
    projwS = np.zeros((128, D), np.float64)
    for h in range(H):
        projwS[32 * h : 32 * h + HS] = proj_W[:, h * HS : (h + 1) * HS].T

    # fold proj into the gate and expert W1 weights (fp64 host math)
    wgf = projwS @ w_gate.astype(np.float64)               # [128, E]
    cb = proj_b.astype(np.float64) @ w_gate.astype(np.float64)   # [E]
    w1f = np.einsum(
        "pd,ehd->eph", projwS, exp_W1.astype(np.float64)
    )                                                      # [E, 128, 600]
    b1f = exp_W1.astype(np.float64) @ proj_b.astype(np.float64) + exp_b1  # [E, 600]

    # b1 bias slices [128, E*5]: column e*5+mc holds b1f[e, h0:h0+hs]
    b1s = np.zeros((128, E * 5), np.float32)
    for e in range(E):
        for mc, (h0, hs) in enumerate(MCH):
            b1s[:hs, e * 5 + mc] = b1f[e, h0 : h0 + hs]

    f16 = np.float16
    common = {
        "wqkv": np.ascontiguousarray(wqkv.T).astype(f16),
        "qkvb": np.ascontiguousarray(qkvb).astype(np.float32),
        "wgf": np.ascontiguousarray(wgf).astype(f16),
        "cb": np.ascontiguousarray(cb[:, None]).astype(np.float32),
        "idm": np.eye(128, dtype=f16),
        "w1t": np.ascontiguousarray(w1f).astype(f16),
        "w2t": np.ascontiguousarray(exp_W2.transpose(0, 2, 1)).astype(f16),
        "b1s": b1s.astype(np.float32),
        "b2s": np.ascontiguousarray(exp_b2[:, None, :]).astype(f16),
    }
    in_maps = []
    for c in range(NCORES):
        xc = x[c * BL : (c + 1) * BL]                  # [8, 300, 768]
        xt = np.ascontiguousarray(
            xc.transpose(2, 0, 1).reshape(DIN, NTOK)
        ).astype(f16)
        in_maps.append({**common, "xt": xt})
    return in_maps


def kernel(x, wh_bias, wh_W, Wq, Wk, Wv, proj_W, proj_b,
           exp_W1, exp_b1, exp_W2, exp_b2, w_gate,
           debug=False, max_phase=9):
    global LAST_RESULT
    x = np.asarray(x, np.float32)
    wh_bias = np.asarray(wh_bias, np.float32)
    wh_W = np.asarray(wh_W, np.float32)
    Wq, Wk, Wv = (np.asarray(w, np.float32) for w in (Wq, Wk, Wv))
    proj_W = np.asarray(proj_W, np.float32)
    proj_b = np.asarray(proj_b, np.float32)
    exp_W1 = np.asarray(exp_W1, np.float32)
    exp_b1 = np.asarray(exp_b1, np.float32)
    exp_W2 = np.asarray(exp_W2, np.float32)
    exp_b2 = np.asarray(exp_b2, np.float32)
    w_gate = np.asarray(w_gate, np.float32)

    b2_nonzero = bool(np.any(exp_b2))
    in_maps = _prep_inputs(x, wh_bias, wh_W, Wq, Wk, Wv, proj_W, proj_b,
                           exp_W1, exp_b1, exp_W2, exp_b2, w_gate)

    nc = _get_module(b2_nonzero, debug=debug, max_phase=max_phase)
    for alloc in nc.m.functions[0].allocations:
        if isinstance(alloc, mybir.MemoryLocationSet) and alloc.kind == "ExternalInput":
            nm = alloc.memorylocations[0].name
            if nm not in in_maps[0]:
                continue  # partition_id etc., supplied by the runner
            got = in_maps[0][nm]
            assert tuple(got.shape) == tuple(alloc.tensor_shape), (
                nm, got.shape, alloc.tensor_shape)
            assert got.dtype == mybir.dt.np(alloc.dtype), (nm, got.dtype)
    res = run_bass_kernel_spmd(nc, in_maps, core_ids=list(range(NCORES)))
    LAST_RESULT = res
    out = np.stack([r["out"] for r in res.results])    # [8, 2400, 300]
    return out.reshape(B, T, D)
